# revision 27
# baseline (speedup 1.0000x reference)
"""Trainium2 Bass kernel for nn_MCAF (dense_transformer).

Strategy: pure data-parallel over 8 NeuronCores (batch 131072 -> 16384/core).
v2: restructured for engine balance + PE p-state:
 - T=1024-col tiles (2 halves x 1024 batch elems = 2048 elems/tile, 8 tiles).
 - Wave structure grouped by ACT table set (exp | absrsqrt | gelu), zigzag
   tile order across waves so engines never drain at wave boundaries.
 - PSUM pools sized to exactly 8 banks per wave with >=2-deep rotation.
 - Front end: eog xo rows DMA'd straight into the y2 tile; h-conv and eog-raw
   contractions merged into one matmul chain; elu via min/exp with bf16 ops.
 - LN: center-matmul -> bf16 evac -> DVE square (4x) -> ones-matmul ->
   ACT Abs_reciprocal_sqrt -> DVE bf16 mul.
 - Classifier bias preloaded into PSUM via DMA; logits DMA'd PSUM->DRAM.
"""

import sys

sys.path.insert(0, "/opt/trn_rl_repo")

import numpy as np
import ml_dtypes

import concourse.bass as bass
import concourse.bacc as bacc
import concourse.tile as tile
from concourse import mybir
from concourse.bass_utils import run_bass_kernel_spmd

F16 = np.float16
F32 = np.float32

B_TOTAL = 131072
N_CORES = 8
B_CORE = B_TOTAL // N_CORES          # 16384
T = 1024                             # batch columns per half-tile
N_TILE = B_CORE // (2 * T)           # 8 tiles of 2 halves x T
AF = mybir.ActivationFunctionType
ALU = mybir.AluOpType
dt = mybir.dt


# ---------------------------------------------------------------- host folding
def _fold_weights(w):
    """Returns (wbf [128,NB] bf16 blob, wf32 [128,NF] f32 bias cols, maps)."""
    eeg_ow = w["eeg_ow"].astype(np.float64)
    wv = w["eeg_inw"][124:186].astype(np.float64)
    bv = w["eeg_inb"][124:186].astype(np.float64)
    Me5 = (eeg_ow @ wv) / 5.0                        # [62,62]
    c_e = eeg_ow @ bv + w["eeg_ob"].astype(np.float64)

    colsb = {}
    bblocks = []

    def addb(name, arr):
        colsb[name] = sum(b.shape[1] for b in bblocks)
        bblocks.append(arr)

    # obar blocks: x320 row g=64*l+c ; M=128 cols: [obar(62) 0 0 | obar(62) 0 0]
    for t in range(3):
        rows = 64 if t == 2 else 128
        blk = np.zeros((128, 128), np.float64)
        for r in range(rows):
            g = 128 * t + r
            c = g % 64
            if c < 62:
                blk[r, 0:62] = Me5[:, c]
                blk[r, 64:126] = Me5[:, c]
            elif g == 62:                            # host plants 1.0 in x320[:,62]
                blk[r, 0:62] = c_e
                blk[r, 64:126] = c_e
        addb(f"ob{t}", blk)

    # h blocks (eeg conv contraction): psHR rows {32h eeg-h} | raw rows later
    cw = w["eeg_cw"].astype(np.float64)              # [32,62,5]
    cwo = w["eog_cw"].astype(np.float64)             # [32,1,33]
    for t in range(3):
        rows = 64 if t == 2 else 128
        blk = np.zeros((128, 32), np.float64)
        for r in range(rows):
            g = 128 * t + r
            l, c = g // 64, g % 64
            if c < 62:
                blk[r, :] = cw[:, c, l]
        addb(f"h{t}", blk)
    # eog raw block (stored at partitions 64:112, read from y2[64:112])
    raw_blk = np.zeros((128, 32), np.float64)
    for l in range(33):
        raw_blk[64 + l, :] = cwo[:, 0, l]
    addb("ogr", raw_blk)

    # eog alpha block (stored at partitions 64:112)
    alpha33 = float(w["eog_inw"][2, 0]) * float(w["eog_ow"][0, 0]) / 33.0
    beta = float(w["eog_inb"][2]) * float(w["eog_ow"][0, 0]) + float(w["eog_ob"][0])
    alp_blk = np.zeros((128, 32), np.float64)
    alp_blk[64:64 + 33, :] = alpha33
    alp_blk[64 + 33, :] = beta                       # host plants 1.0 in xo row 33
    addb("alp", alp_blk)

    # fused (ef,of)->feat weights; psF rows [feat-h0 | feat-h1]
    fw = w["fus_w"].astype(np.float64)               # [64,128]
    W_e = fw[:, :64] @ w["eeg_fw"].astype(np.float64)    # [64,32]
    W_o = fw[:, 64:] @ w["eog_fw"].astype(np.float64)    # [64,32]
    elu_blk = np.zeros((128, 128), np.float64)
    elu_blk[0:32, 0:64] = W_e.T                      # eeg_h0 -> feat h0
    elu_blk[32:64, 64:128] = W_e.T                   # eeg_h1 -> feat h1
    elu_blk[64:96, 0:64] = W_o.T                     # og_h0  -> feat h0
    elu_blk[96:128, 64:128] = W_o.T                  # og_h1  -> feat h1
    addb("elu", elu_blk)

    def bdiag(blk):                                  # block-diag [128,128]
        out = np.zeros((128, 128))
        out[0:64, 0:64] = blk
        out[64:128, 64:128] = blk
        return out

    C = np.eye(64) - 1.0 / 64.0
    addb("center", bdiag(C))
    addb("ones64", bdiag(np.ones((64, 64))))

    pe0 = (np.arange(64) % 2).astype(np.float64)
    b_feat = (fw[:, :64] @ w["eeg_fb"].astype(np.float64)
              + fw[:, 64:] @ w["eog_fb"].astype(np.float64)
              + w["fus_b"].astype(np.float64) + pe0
              - W_e.sum(axis=1) - W_o.sum(axis=1))   # fold elu's (e'-1)

    lay = []
    for i in range(2):
        s1 = w["tl_ln1_s"][i].astype(np.float64)
        b1v = w["tl_ln1_b"][i].astype(np.float64)
        Wvo = w["tl_ow"][i].astype(np.float64) @ w["tl_inw"][i, 128:192].astype(np.float64)
        bvo = (w["tl_ow"][i].astype(np.float64) @ w["tl_inb"][i, 128:192].astype(np.float64)
               + w["tl_ob"][i].astype(np.float64))
        Wvo_s = Wvo * s1[None, :]
        bvo_t = Wvo @ b1v + bvo
        s2 = w["tl_ln2_s"][i].astype(np.float64)
        b2v = w["tl_ln2_b"][i].astype(np.float64)
        W1 = w["tl_w1"][i].astype(np.float64)        # [256,64]
        W1_s = W1 * s2[None, :]
        b1g = W1 @ b2v + w["tl_b1"][i].astype(np.float64)   # [256]
        W2 = w["tl_w2"][i].astype(np.float64)        # [64,256]
        b2c = w["tl_b2"][i].astype(np.float64)
        lay.append((Wvo_s, bvo_t, W1_s, b1g, W2, b2c))
        addb(f"attn{i}", bdiag(Wvo_s.T))
        m1 = W1_s.T                                  # [64,256]
        w2t = W2.T                                   # [256,64]
        for q in range(4):
            addb(f"mlp1q{i}_{q}", bdiag(m1[:, 64 * q:64 * q + 64]))
            addb(f"mlp2q{i}_{q}", bdiag(w2t[64 * q:64 * q + 64, :]))

    fn_s = w["fn_s"].astype(np.float64)
    fn_b = w["fn_b"].astype(np.float64)
    cls_w = w["cls_w"].astype(np.float64)
    cls_s = cls_w * fn_s[None, :]                    # [3,64]
    b_cls = cls_w @ fn_b + w["cls_b"].astype(np.float64)
    csT = cls_s.T                                    # [64,3]
    clsblk = np.zeros((128, 6))
    clsblk[0:64, 0:3] = csT
    clsblk[64:128, 3:6] = csT
    addb("cls", clsblk)

    wbf = np.concatenate(bblocks, axis=1)

    # --- f32 bias columns ---
    cols = {}
    blocks = []

    def add(name, arr):
        cols[name] = sum(b.shape[1] for b in blocks)
        blocks.append(arr)

    def col(vals128):
        return np.asarray(vals128, np.float64).reshape(128, 1)

    cb_e = w["eeg_cb"].astype(np.float64)
    cb_o = w["eog_cb"].astype(np.float64)
    add("cbcol", col(np.concatenate([cb_e, cb_e, cb_o, cb_o])))
    add("bfeat", col(np.concatenate([b_feat, b_feat])))
    for i in range(2):
        add(f"bvo{i}", col(np.concatenate([lay[i][1], lay[i][1]])))
        for q in range(4):
            add(f"b1gq{i}_{q}", col(np.concatenate([lay[i][3][64 * q:64 * q + 64]] * 2)))
        add(f"b2c{i}", col(np.concatenate([lay[i][5], lay[i][5]])))
    add("eps", col(np.full(128, 1e-5)))
    add("zero", col(np.zeros(128)))
    bc6 = np.zeros(128)
    bc6[0:3] = b_cls
    bc6[3:6] = b_cls
    add("bcls6", col(bc6))

    wf32 = np.concatenate(blocks, axis=1)
    return wbf.astype(F16), wf32.astype(F32), cols, colsb


# ---------------------------------------------------------------- device build
_CACHE = {}


def _build(nbf, nf32, cols, colsb):
    nc = bacc.Bacc("TRN2", target_bir_lowering=False, debug=False)
    xe_d = nc.dram_tensor("xe", [320, B_CORE], dt.float16, kind="ExternalInput")
    xo_d = nc.dram_tensor("xo", [48, B_CORE], dt.float16, kind="ExternalInput")
    wbf_d = nc.dram_tensor("wbf", [128, nbf], dt.float16, kind="ExternalInput")
    wf_d = nc.dram_tensor("wf32", [128, nf32], dt.float32, kind="ExternalInput")
    y_d = nc.dram_tensor("y_fm", [6, N_TILE * T], dt.float32, kind="ExternalOutput")

    # persistent sbuf arrays
    wbf_sb = nc.alloc_sbuf_tensor("wbf_sb", [128, nbf], dt.float16).ap()
    wf_sb = nc.alloc_sbuf_tensor("wf_sb", [128, nf32], dt.float32).ap()
    featA = nc.alloc_sbuf_tensor("featA", [128, N_TILE * T], dt.float16).ap()
    featB = nc.alloc_sbuf_tensor("featB", [128, N_TILE * T], dt.float16).ap()
    xn_sb = nc.alloc_sbuf_tensor("xn_sb", [128, N_TILE * T], dt.float16).ap()

    def Wb(name, r0=0, r1=128, n=128):
        c0 = colsb[name]
        return wbf_sb[r0:r1, c0:c0 + n]

    def Wc(name, r0=0, r1=128):                      # bias columns
        return wf_sb[r0:r1, cols[name]:cols[name] + 1]

    def fwd(w):                                      # zigzag tile order per wave
        return range(N_TILE) if w % 2 == 0 else range(N_TILE - 1, -1, -1)

    wno = 0

    with tile.TileContext(nc) as tc:
        nc.sync.dma_start(wbf_sb, wbf_d.ap())
        nc.sync.dma_start(wf_sb, wf_d.ap())

        # =================== WAVE 1: front end (ACT table: exp) ===============
        with tc.tile_pool(name="w1x", bufs=2) as xp, \
             tc.tile_pool(name="w1s", bufs=2) as sb, \
             tc.tile_pool(name="w1p", bufs=1, space="PSUM") as ps:
            for k in fwd(wno):
                c0 = k * 2 * T                       # first batch col of tile
                xt = xp.tile([128, 3 * 2 * T], dt.float16, tag="xt")
                nc.sync.dma_start(xt[:, 0:2 * T], xe_d.ap()[0:128, c0:c0 + 2 * T])
                nc.sync.dma_start(xt[:, 2 * T:4 * T], xe_d.ap()[128:256, c0:c0 + 2 * T])
                nc.sync.dma_start(xt[0:64, 4 * T:6 * T], xe_d.ap()[256:320, c0:c0 + 2 * T])
                y2s = []
                for h in range(2):
                    y2 = xp.tile([112, T], dt.float16, tag=f"y2{h}")
                    nc.sync.dma_start(y2[64:112], xo_d.ap()[:, c0 + h * T:c0 + (h + 1) * T])
                    y2s.append(y2)

                psHR = ps.tile([128, T], dt.float32, tag="HR", bufs=1)
                psA = ps.tile([128, T], dt.float32, tag="A")
                for h in range(2):
                    ob2 = sb.tile([128, T], dt.float16, tag=f"ob2{h}")
                    # obar: 3 chunks x 2 slices, chunk-major for weight reuse
                    psobs = [ps.tile([128, 512], dt.float32, tag="OB", bufs=2,
                                     name=f"psob{h}{j}") for j in range(2)]
                    for t in range(3):
                        kk = 64 if t == 2 else 128
                        for s in range(2):
                            mv = xt[0:kk, 2 * T * t + h * T + 512 * s:
                                    2 * T * t + h * T + 512 * s + 512]
                            nc.tensor.matmul(psobs[s], Wb(f"ob{t}")[0:kk], mv,
                                             start=(t == 0), stop=(t == 2))
                    for s in range(2):
                        nc.scalar.activation(ob2[:, 512 * s:512 * s + 512], psobs[s],
                                             AF.Identity, bias=Wc("zero"))
                    # y = x * obar  (bf16, 4x DVE)
                    y0 = sb.tile([128, T], dt.float16, tag=f"y0{h}")
                    y1 = sb.tile([128, T], dt.float16, tag=f"y1{h}")
                    nc.vector.tensor_mul(y0, xt[:, h * T:(h + 1) * T], ob2)
                    nc.vector.tensor_mul(y1, xt[:, 2 * T + h * T:2 * T + (h + 1) * T], ob2)
                    nc.vector.tensor_mul(y2s[h][0:64],
                                         xt[0:64, 4 * T + h * T:4 * T + (h + 1) * T],
                                         ob2[0:64])
                    # eeg h chain -> psHR rows 32h:32h+32
                    for t in range(3):
                        kk = 64 if t == 2 else 128
                        for s in range(2):
                            if t == 2:
                                mv = y2s[h][0:64, 512 * s:512 * s + 512]
                            else:
                                yy = y0 if t == 0 else y1
                                mv = yy[:, 512 * s:512 * s + 512]
                            nc.tensor.matmul(psHR[32 * h:32 * h + 32,
                                                  512 * s:512 * s + 512],
                                             Wb(f"h{t}", 0, kk, n=32), mv,
                                             start=(t == 0), stop=(t == 2))
                # eog alpha / raw MMs (psR reuses the OB psum ring)
                psRs = [ps.tile([128, 512], dt.float32, tag="OB", bufs=2,
                                name=f"psr{j}") for j in range(2)]
                for h in range(2):
                    o2 = 64 + 32 * h
                    for s in range(2):
                        nc.tensor.matmul(psA[o2:o2 + 32, 512 * s:512 * s + 512],
                                         Wb("alp", 64, 112, n=32),
                                         y2s[h][64:112, 512 * s:512 * s + 512],
                                         tile_position=(64, o2))
                        nc.tensor.matmul(psRs[s][o2:o2 + 32],
                                         Wb("ogr", 64, 112, n=32),
                                         y2s[h][64:112, 512 * s:512 * s + 512],
                                         tile_position=(64, o2))
                # og = raw * (alpha*mean + beta): evac alpha, mult into psHR
                o2a = sb.tile([128, T], dt.float16, tag="o2a")
                nc.scalar.activation(o2a[64:128], psA[64:128], AF.Identity,
                                     bias=Wc("zero", 64, 128))
                for s in range(2):
                    nc.vector.tensor_mul(psHR[64:128, 512 * s:512 * s + 512],
                                         psRs[s][64:128],
                                         o2a[64:128, 512 * s:512 * s + 512])
                # elu(z)+1 = max(z,0) + exp(min(z,0)); -1 folded into bfeat
                r1 = sb.tile([128, T], dt.float16, tag="r1")
                sm = sb.tile([128, T], dt.float16, tag="sm")
                e1 = sb.tile([128, T], dt.float16, tag="e1")
                eluT = sb.tile([128, T], dt.float16, tag="elu")
                nc.scalar.activation(r1, psHR, AF.Relu, bias=Wc("cbcol"))
                nc.vector.tensor_scalar(sm, psHR, Wc("cbcol"), 0.0, ALU.add, ALU.min)
                nc.scalar.activation(e1, sm, AF.Exp, bias=Wc("zero"))
                nc.vector.tensor_add(eluT, r1, e1)
                # feat = W_elu.T @ elu (+b_feat)
                for s in range(2):
                    psF = ps.tile([128, 512], dt.float32, tag="F", bufs=2)
                    nc.tensor.matmul(psF, Wb("elu"), eluT[:, 512 * s:512 * s + 512])
                    nc.scalar.activation(featA[:, c0 // 2 + 512 * s:c0 // 2 + 512 * s + 512],
                                         psF, AF.Identity, bias=Wc("bfeat"))
        wno += 1

        # =================== transformer ===================
        def ln_chain(sb, ps, src_ap, xn_out_ap):
            """LN on one [128,T] tile: ACT evac -> fp16 DVE sq -> ones-MM ->
            AbsRsqrt -> fp16 DVE mul. src_ap is fp16 SBUF [128, T]."""
            psXC = ps.tile([128, T], dt.float32, tag="XC", bufs=2, name="psxc")
            for s in range(2):
                nc.tensor.matmul(psXC[:, 512 * s:512 * s + 512], Wb("center"),
                                 src_ap[:, 512 * s:512 * s + 512])
            xc = sb.tile([128, T], dt.float16, tag="xc", bufs=2, name="xc")
            nc.scalar.activation(xc, psXC, AF.Identity, bias=Wc("zero"))
            sq = sb.tile([128, T], dt.float16, tag="sq", bufs=2, name="sq")
            nc.vector.tensor_mul(sq, xc, xc)
            psV = ps.tile([128, T], dt.float32, tag="V", bufs=1, name="psv")
            for s in range(2):
                nc.tensor.matmul(psV[:, 512 * s:512 * s + 512], Wb("ones64"),
                                 sq[:, 512 * s:512 * s + 512])
            rstd = sb.tile([128, T], dt.float16, tag="rs", bufs=2, name="rstd")
            nc.scalar.activation(rstd, psV, AF.Abs_reciprocal_sqrt,
                                 bias=Wc("eps"), scale=1.0 / 64.0)
            nc.vector.tensor_mul(xn_out_ap, xc, rstd)

        for i in range(2):
            # ---- LN1 + attn + residual + LN2 (table: abs_reciprocal_sqrt) ----
            tc.no_sync_barrier()
            with tc.tile_pool(name=f"at{i}s", bufs=2) as sb, \
                 tc.tile_pool(name=f"at{i}p", bufs=1, space="PSUM") as ps:
                for k in fwd(wno):
                    sl = slice(k * T, (k + 1) * T)
                    xn1 = sb.tile([128, T], dt.float16, tag="xn1", bufs=2)
                    ln_chain(sb, ps, featA[:, sl], xn1)
                    psF2 = ps.tile([128, T], dt.float32, tag="F2", bufs=1)
                    for s in range(2):
                        nc.tensor.matmul(psF2[:, 512 * s:512 * s + 512], Wb(f"attn{i}"),
                                         xn1[:, 512 * s:512 * s + 512])
                    nc.vector.scalar_tensor_tensor(
                        featB[:, sl], psF2, Wc(f"bvo{i}"), featA[:, sl],
                        ALU.add, ALU.add)
                    ln_chain(sb, ps, featB[:, sl], xn_sb[:, sl])
            wno += 1
            # ---- MLP (ACT table: gelu) ----
            tc.no_sync_barrier()
            with tc.tile_pool(name=f"ml{i}s", bufs=6) as sb, \
                 tc.tile_pool(name=f"ml{i}p", bufs=2, space="PSUM") as ps:
                for k in fwd(wno):
                    sl = slice(k * T, (k + 1) * T)
                    gsb = []
                    for q in range(4):
                        gp = ps.tile([128, T], dt.float32, tag="G")
                        for s in range(2):
                            nc.tensor.matmul(gp[:, 512 * s:512 * s + 512],
                                             Wb(f"mlp1q{i}_{q}"),
                                             xn_sb[:, k * T + 512 * s:k * T + 512 * s + 512])
                        g = sb.tile([128, T], dt.float16, tag=f"g{q}")
                        nc.scalar.activation(g, gp, AF.Gelu, bias=Wc(f"b1gq{i}_{q}"))
                        gsb.append(g)
                    psF3 = ps.tile([128, T], dt.float32, tag="F3")
                    for q in range(4):
                        for s in range(2):
                            nc.tensor.matmul(psF3[:, 512 * s:512 * s + 512],
                                             Wb(f"mlp2q{i}_{q}"),
                                             gsb[q][:, 512 * s:512 * s + 512],
                                             start=(q == 0), stop=(q == 3))
                    nc.vector.scalar_tensor_tensor(
                        featA[:, sl], psF3, Wc(f"b2c{i}"), featB[:, sl],
                        ALU.add, ALU.add)
            wno += 1

        # ---- final LN + classifier (abs_reciprocal_sqrt) ----
        tc.no_sync_barrier()
        with tc.tile_pool(name="clss", bufs=2) as sb, \
             tc.tile_pool(name="clsp", bufs=1, space="PSUM") as ps:
            for k in fwd(wno):
                xn3 = sb.tile([128, T], dt.float16, tag="xn3", bufs=2)
                ln_chain(sb, ps, featA[:, k * T:(k + 1) * T], xn3)
                psO = ps.tile([6, T], dt.float32, tag="O", bufs=1)
                for s in range(2):
                    nc.tensor.matmul(psO[:, 512 * s:512 * s + 512], Wb("cls", n=6),
                                     xn3[:, 512 * s:512 * s + 512])
                osb = sb.tile([6, T], dt.float32, tag="osb")
                nc.vector.tensor_scalar_add(osb, psO, Wc("bcls6", 0, 6))
                nc.sync.dma_start(y_d.ap()[:, k * T:(k + 1) * T], osb)
        wno += 1

    nc.compile()
    return nc


# ---------------------------------------------------------------- entry point
def _prep_x(w):
    """Host-side: build transposed bf16 input blobs xe [320,B], xo [48,B]."""
    eeg = w["eeg"].astype(F32)                       # [B, 62, 5]
    xeT = np.zeros((320, B_TOTAL), F32)
    xeT.reshape(5, 64, B_TOTAL)[:, 0:62, :] = eeg.transpose(2, 1, 0)
    xeT[62] = 1.0
    xoT = np.zeros((48, B_TOTAL), F32)
    xoT[0:33] = w["eog"].astype(F32)[:, 0, :].T
    xoT[33] = 1.0                                    # beta bias row
    return xeT.astype(F16), xoT.astype(F16)


def _make_in_maps(w):
    wbf, wf32, cols, colsb = _fold_weights(w)
    xeT, xoT = _prep_x(w)
    key = ("prog", wbf.shape[1], wf32.shape[1])
    in_maps = []
    for k in range(N_CORES):
        in_maps.append({
            "xe": np.ascontiguousarray(xeT[:, k * B_CORE:(k + 1) * B_CORE]),
            "xo": np.ascontiguousarray(xoT[:, k * B_CORE:(k + 1) * B_CORE]),
            "wbf": wbf, "wf32": wf32,
        })
    return key, in_maps, (wbf.shape[1], wf32.shape[1], cols, colsb)


def _unshard(res):
    out = np.empty((B_TOTAL, 3), F32)
    for k in range(N_CORES):
        y = res.results[k]["y_fm"].reshape(2, 3, N_TILE, T)
        out[k * B_CORE:(k + 1) * B_CORE] = (
            y.transpose(2, 0, 3, 1).reshape(B_CORE, 3))
    return out


def kernel(**inputs):
    w = {k: np.asarray(v) for k, v in inputs.items()}
    key, in_maps, (nbf, nf32, cols, colsb) = _make_in_maps(w)
    if key not in _CACHE:
        _CACHE[key] = _build(nbf, nf32, cols, colsb)
    nc = _CACHE[key]
    res = run_bass_kernel_spmd(nc, in_maps, core_ids=list(range(N_CORES)))
    return _unshard(res)


if __name__ == "__main__":
    import reference
    ins = {k: np.asarray(v) for k, v in reference.setup_inputs().items()}
    got = kernel(**ins)
    exp = np.asarray(reference.reference(**ins))
    err = np.abs(got - exp).max() / (np.abs(exp).max() + 1e-9)
    print("Relative error:", err)


# revision 31
# speedup vs baseline: 1.1727x; 1.1727x over previous
"""Trainium2 Bass kernel for nn_MCAF (dense_transformer).

Strategy: pure data-parallel over 8 NeuronCores (batch 131072 -> 16384/core).
v2: restructured for engine balance + PE p-state:
 - T=1024-col tiles (2 halves x 1024 batch elems = 2048 elems/tile, 8 tiles).
 - Wave structure grouped by ACT table set (exp | absrsqrt | gelu), zigzag
   tile order across waves so engines never drain at wave boundaries.
 - PSUM pools sized to exactly 8 banks per wave with >=2-deep rotation.
 - Front end: eog xo rows DMA'd straight into the y2 tile; h-conv and eog-raw
   contractions merged into one matmul chain; elu via min/exp with bf16 ops.
 - LN: center-matmul -> bf16 evac -> DVE square (4x) -> ones-matmul ->
   ACT Abs_reciprocal_sqrt -> DVE bf16 mul.
 - Classifier bias preloaded into PSUM via DMA; logits DMA'd PSUM->DRAM.
"""

import sys

sys.path.insert(0, "/opt/trn_rl_repo")

import numpy as np
import ml_dtypes

import concourse.bass as bass
import concourse.bacc as bacc
import concourse.tile as tile
from concourse import mybir
from concourse.bass_utils import run_bass_kernel_spmd

F16 = np.float16
F32 = np.float32

B_TOTAL = 131072
N_CORES = 8
B_CORE = B_TOTAL // N_CORES          # 16384
T = 1024                             # batch columns per half-tile
N_TILE = B_CORE // (2 * T)           # 8 tiles of 2 halves x T
AF = mybir.ActivationFunctionType
ALU = mybir.AluOpType
dt = mybir.dt


# ---------------------------------------------------------------- host folding
def _fold_weights(w):
    """Returns (wbf [128,NB] bf16 blob, wf32 [128,NF] f32 bias cols, maps)."""
    eeg_ow = w["eeg_ow"].astype(np.float64)
    wv = w["eeg_inw"][124:186].astype(np.float64)
    bv = w["eeg_inb"][124:186].astype(np.float64)
    Me5 = (eeg_ow @ wv) / 5.0                        # [62,62]
    c_e = eeg_ow @ bv + w["eeg_ob"].astype(np.float64)

    colsb = {}
    bblocks = []

    def addb(name, arr):
        colsb[name] = sum(b.shape[1] for b in bblocks)
        bblocks.append(arr)

    # obar blocks: x320 row g=64*l+c ; M=128 cols: [obar(62) 0 0 | obar(62) 0 0]
    for t in range(3):
        rows = 64 if t == 2 else 128
        blk = np.zeros((128, 128), np.float64)
        for r in range(rows):
            g = 128 * t + r
            c = g % 64
            if c < 62:
                blk[r, 0:62] = Me5[:, c]
                blk[r, 64:126] = Me5[:, c]
            elif g == 62:                            # host plants 1.0 in x320[:,62]
                blk[r, 0:62] = c_e
                blk[r, 64:126] = c_e
        addb(f"ob{t}", blk)

    # h blocks (eeg conv contraction): psHR rows {32h eeg-h} | raw rows later
    cw = w["eeg_cw"].astype(np.float64)              # [32,62,5]
    cwo = w["eog_cw"].astype(np.float64)             # [32,1,33]
    for t in range(3):
        rows = 64 if t == 2 else 128
        blk = np.zeros((128, 32), np.float64)
        for r in range(rows):
            g = 128 * t + r
            l, c = g // 64, g % 64
            if c < 62:
                blk[r, :] = cw[:, c, l]
        addb(f"h{t}", blk)
    # eog raw block (stored at partitions 64:112, read from y2[64:112])
    raw_blk = np.zeros((128, 32), np.float64)
    for l in range(33):
        raw_blk[64 + l, :] = cwo[:, 0, l]
    addb("ogr", raw_blk)

    # eog alpha block (stored at partitions 64:112)
    alpha33 = float(w["eog_inw"][2, 0]) * float(w["eog_ow"][0, 0]) / 33.0
    beta = float(w["eog_inb"][2]) * float(w["eog_ow"][0, 0]) + float(w["eog_ob"][0])
    alp_blk = np.zeros((128, 32), np.float64)
    alp_blk[64:64 + 33, :] = alpha33
    alp_blk[64 + 33, :] = beta                       # host plants 1.0 in xo row 33
    addb("alp", alp_blk)

    # fused (ef,of)->feat weights; psF rows [feat-h0 | feat-h1]
    fw = w["fus_w"].astype(np.float64)               # [64,128]
    W_e = fw[:, :64] @ w["eeg_fw"].astype(np.float64)    # [64,32]
    W_o = fw[:, 64:] @ w["eog_fw"].astype(np.float64)    # [64,32]
    elu_blk = np.zeros((128, 128), np.float64)
    elu_blk[0:32, 0:64] = W_e.T                      # eeg_h0 -> feat h0
    elu_blk[32:64, 64:128] = W_e.T                   # eeg_h1 -> feat h1
    elu_blk[64:96, 0:64] = W_o.T                     # og_h0  -> feat h0
    elu_blk[96:128, 64:128] = W_o.T                  # og_h1  -> feat h1
    addb("elu", elu_blk)

    def bdiag(blk):                                  # block-diag [128,128]
        out = np.zeros((128, 128))
        out[0:64, 0:64] = blk
        out[64:128, 64:128] = blk
        return out

    C = np.eye(64) - 1.0 / 64.0
    addb("center", bdiag(C))
    addb("ones64", bdiag(np.ones((64, 64))))

    pe0 = (np.arange(64) % 2).astype(np.float64)
    b_feat = (fw[:, :64] @ w["eeg_fb"].astype(np.float64)
              + fw[:, 64:] @ w["eog_fb"].astype(np.float64)
              + w["fus_b"].astype(np.float64) + pe0
              - W_e.sum(axis=1) - W_o.sum(axis=1))   # fold elu's (e'-1)

    lay = []
    for i in range(2):
        s1 = w["tl_ln1_s"][i].astype(np.float64)
        b1v = w["tl_ln1_b"][i].astype(np.float64)
        Wvo = w["tl_ow"][i].astype(np.float64) @ w["tl_inw"][i, 128:192].astype(np.float64)
        bvo = (w["tl_ow"][i].astype(np.float64) @ w["tl_inb"][i, 128:192].astype(np.float64)
               + w["tl_ob"][i].astype(np.float64))
        Wvo_s = Wvo * s1[None, :]
        bvo_t = Wvo @ b1v + bvo
        s2 = w["tl_ln2_s"][i].astype(np.float64)
        b2v = w["tl_ln2_b"][i].astype(np.float64)
        W1 = w["tl_w1"][i].astype(np.float64)        # [256,64]
        W1_s = W1 * s2[None, :]
        b1g = W1 @ b2v + w["tl_b1"][i].astype(np.float64)   # [256]
        W2 = w["tl_w2"][i].astype(np.float64)        # [64,256]
        b2c = w["tl_b2"][i].astype(np.float64)
        lay.append((Wvo_s, bvo_t, W1_s, b1g, W2, b2c))
        addb(f"attn{i}", bdiag(Wvo_s.T))
        m1 = W1_s.T                                  # [64,256]
        w2t = W2.T                                   # [256,64]
        for q in range(4):
            addb(f"mlp1q{i}_{q}", bdiag(m1[:, 64 * q:64 * q + 64]))
            addb(f"mlp2q{i}_{q}", bdiag(w2t[64 * q:64 * q + 64, :]))

    fn_s = w["fn_s"].astype(np.float64)
    fn_b = w["fn_b"].astype(np.float64)
    cls_w = w["cls_w"].astype(np.float64)
    cls_s = cls_w * fn_s[None, :]                    # [3,64]
    b_cls = cls_w @ fn_b + w["cls_b"].astype(np.float64)
    csT = cls_s.T                                    # [64,3]
    clsblk = np.zeros((128, 6))
    clsblk[0:64, 0:3] = csT
    clsblk[64:128, 3:6] = csT
    addb("cls", clsblk)

    wbf = np.concatenate(bblocks, axis=1)

    # --- f32 bias columns ---
    cols = {}
    blocks = []

    def add(name, arr):
        cols[name] = sum(b.shape[1] for b in blocks)
        blocks.append(arr)

    def col(vals128):
        return np.asarray(vals128, np.float64).reshape(128, 1)

    cb_e = w["eeg_cb"].astype(np.float64)
    cb_o = w["eog_cb"].astype(np.float64)
    add("cbcol", col(np.concatenate([cb_e, cb_e, cb_o, cb_o])))
    add("bfeat", col(np.concatenate([b_feat, b_feat])))
    for i in range(2):
        add(f"bvo{i}", col(np.concatenate([lay[i][1], lay[i][1]])))
        for q in range(4):
            add(f"b1gq{i}_{q}", col(np.concatenate([lay[i][3][64 * q:64 * q + 64]] * 2)))
        add(f"b2c{i}", col(np.concatenate([lay[i][5], lay[i][5]])))
    add("eps", col(np.full(128, 1e-5)))
    add("zero", col(np.zeros(128)))
    bc6 = np.zeros(128)
    bc6[0:3] = b_cls
    bc6[3:6] = b_cls
    add("bcls6", col(bc6))

    wf32 = np.concatenate(blocks, axis=1)
    return wbf.astype(F16), wf32.astype(F32), cols, colsb


# ---------------------------------------------------------------- device build
_CACHE = {}


def _build(nbf, nf32, cols, colsb):
    nc = bacc.Bacc("TRN2", target_bir_lowering=False, debug=False)
    xe_d = nc.dram_tensor("xe", [320, B_CORE], dt.float16, kind="ExternalInput")
    xo_d = nc.dram_tensor("xo", [48, B_CORE], dt.float16, kind="ExternalInput")
    wbf_d = nc.dram_tensor("wbf", [128, nbf], dt.float16, kind="ExternalInput")
    wf_d = nc.dram_tensor("wf32", [128, nf32], dt.float32, kind="ExternalInput")
    y_d = nc.dram_tensor("y_fm", [6, N_TILE * T], dt.float32, kind="ExternalOutput")

    # persistent sbuf arrays
    wbf_sb = nc.alloc_sbuf_tensor("wbf_sb", [128, nbf], dt.float16).ap()
    wf_sb = nc.alloc_sbuf_tensor("wf_sb", [128, nf32], dt.float32).ap()
    featA = nc.alloc_sbuf_tensor("featA", [128, N_TILE * T], dt.float16).ap()
    featB = nc.alloc_sbuf_tensor("featB", [128, N_TILE * T], dt.float16).ap()
    xn_sb = nc.alloc_sbuf_tensor("xn_sb", [128, N_TILE * T], dt.float16).ap()

    def Wb(name, r0=0, r1=128, n=128):
        c0 = colsb[name]
        return wbf_sb[r0:r1, c0:c0 + n]

    def Wc(name, r0=0, r1=128):                      # bias columns
        return wf_sb[r0:r1, cols[name]:cols[name] + 1]

    def fwd(w):                                      # zigzag tile order per wave
        return range(N_TILE) if w % 2 == 0 else range(N_TILE - 1, -1, -1)

    wno = 0

    with tile.TileContext(nc) as tc:
        nc.sync.dma_start(wbf_sb, wbf_d.ap())
        nc.sync.dma_start(wf_sb, wf_d.ap())

        # =================== WAVE 1: front end (ACT table: exp) ===============
        with tc.tile_pool(name="w1x", bufs=2) as xp, \
             tc.tile_pool(name="w1s", bufs=2) as sb, \
             tc.tile_pool(name="w1p", bufs=1, space="PSUM") as ps:
            for k in fwd(wno):
                c0 = k * 2 * T                       # first batch col of tile
                xt = xp.tile([128, 3 * 2 * T], dt.float16, tag="xt")
                nc.sync.dma_start(xt[:, 0:2 * T], xe_d.ap()[0:128, c0:c0 + 2 * T])
                nc.sync.dma_start(xt[:, 2 * T:4 * T], xe_d.ap()[128:256, c0:c0 + 2 * T])
                nc.sync.dma_start(xt[0:64, 4 * T:6 * T], xe_d.ap()[256:320, c0:c0 + 2 * T])
                y2s = []
                for h in range(2):
                    y2 = xp.tile([112, T], dt.float16, tag=f"y2{h}")
                    nc.sync.dma_start(y2[64:112], xo_d.ap()[:, c0 + h * T:c0 + (h + 1) * T])
                    y2s.append(y2)

                psHR = ps.tile([128, T], dt.float32, tag="HR", bufs=1)
                psA = ps.tile([128, T], dt.float32, tag="A")
                for h in range(2):
                    ob2 = sb.tile([128, T], dt.float16, tag=f"ob2{h}")
                    # obar: 3 chunks x 2 slices, chunk-major for weight reuse
                    psobs = [ps.tile([128, 512], dt.float32, tag="OB", bufs=2,
                                     name=f"psob{h}{j}") for j in range(2)]
                    for t in range(3):
                        kk = 64 if t == 2 else 128
                        for s in range(2):
                            mv = xt[0:kk, 2 * T * t + h * T + 512 * s:
                                    2 * T * t + h * T + 512 * s + 512]
                            nc.tensor.matmul(psobs[s], Wb(f"ob{t}")[0:kk], mv,
                                             start=(t == 0), stop=(t == 2))
                    for s in range(2):
                        nc.scalar.activation(ob2[:, 512 * s:512 * s + 512], psobs[s],
                                             AF.Identity, bias=Wc("zero"))
                    # y = x * obar  (bf16, 4x DVE)
                    y0 = sb.tile([128, T], dt.float16, tag=f"y0{h}")
                    y1 = sb.tile([128, T], dt.float16, tag=f"y1{h}")
                    nc.vector.tensor_mul(y0, xt[:, h * T:(h + 1) * T], ob2)
                    nc.vector.tensor_mul(y1, xt[:, 2 * T + h * T:2 * T + (h + 1) * T], ob2)
                    nc.vector.tensor_mul(y2s[h][0:64],
                                         xt[0:64, 4 * T + h * T:4 * T + (h + 1) * T],
                                         ob2[0:64])
                    # eeg h chain -> psHR rows 32h:32h+32
                    for t in range(3):
                        kk = 64 if t == 2 else 128
                        for s in range(2):
                            if t == 2:
                                mv = y2s[h][0:64, 512 * s:512 * s + 512]
                            else:
                                yy = y0 if t == 0 else y1
                                mv = yy[:, 512 * s:512 * s + 512]
                            nc.tensor.matmul(psHR[32 * h:32 * h + 32,
                                                  512 * s:512 * s + 512],
                                             Wb(f"h{t}", 0, kk, n=32), mv,
                                             start=(t == 0), stop=(t == 2))
                # eog alpha / raw MMs (psR reuses the OB psum ring)
                psRs = [ps.tile([128, 512], dt.float32, tag="OB", bufs=2,
                                name=f"psr{j}") for j in range(2)]
                for h in range(2):
                    o2 = 64 + 32 * h
                    for s in range(2):
                        nc.tensor.matmul(psA[o2:o2 + 32, 512 * s:512 * s + 512],
                                         Wb("alp", 64, 112, n=32),
                                         y2s[h][64:112, 512 * s:512 * s + 512],
                                         tile_position=(64, o2))
                        nc.tensor.matmul(psRs[s][o2:o2 + 32],
                                         Wb("ogr", 64, 112, n=32),
                                         y2s[h][64:112, 512 * s:512 * s + 512],
                                         tile_position=(64, o2))
                # og = raw * (alpha*mean + beta): evac alpha, mult into psHR
                o2a = sb.tile([128, T], dt.float16, tag="o2a")
                nc.scalar.activation(o2a[64:128], psA[64:128], AF.Identity,
                                     bias=Wc("zero", 64, 128))
                for s in range(2):
                    nc.vector.tensor_mul(psHR[64:128, 512 * s:512 * s + 512],
                                         psRs[s][64:128],
                                         o2a[64:128, 512 * s:512 * s + 512])
                # elu(z)+1 = max(z,0) + exp(min(z,0)); -1 folded into bfeat
                r1 = sb.tile([128, T], dt.float16, tag="r1")
                sm = sb.tile([128, T], dt.float16, tag="sm")
                e1 = sb.tile([128, T], dt.float16, tag="e1")
                eluT = sb.tile([128, T], dt.float16, tag="elu")
                nc.scalar.activation(r1, psHR, AF.Relu, bias=Wc("cbcol"))
                nc.vector.tensor_scalar(sm, psHR, Wc("cbcol"), 0.0, ALU.add, ALU.min)
                nc.scalar.activation(e1, sm, AF.Exp, bias=Wc("zero"))
                nc.vector.tensor_add(eluT, r1, e1)
                # feat = W_elu.T @ elu (+b_feat)
                for s in range(2):
                    psF = ps.tile([128, 512], dt.float32, tag="F", bufs=2)
                    nc.tensor.matmul(psF, Wb("elu"), eluT[:, 512 * s:512 * s + 512])
                    nc.scalar.activation(featA[:, c0 // 2 + 512 * s:c0 // 2 + 512 * s + 512],
                                         psF, AF.Identity, bias=Wc("bfeat"))
        wno += 1

        # =================== transformer ===================
        def ln_chain(sb, ps, src_ap, xn_out_ap, vbufs=2):
            """LN on one [128,T] tile: ACT evac -> fp16 DVE sq -> ones-MM ->
            AbsRsqrt -> fp16 DVE mul. src_ap is fp16 SBUF [128, T]."""
            psXC = ps.tile([128, T], dt.float32, tag="XC", bufs=2, name="psxc")
            for s in range(2):
                nc.tensor.matmul(psXC[:, 512 * s:512 * s + 512], Wb("center"),
                                 src_ap[:, 512 * s:512 * s + 512])
            xc = sb.tile([128, T], dt.float16, tag="xc", bufs=2, name="xc")
            nc.scalar.activation(xc, psXC, AF.Identity, bias=Wc("zero"))
            sq = sb.tile([128, T], dt.float16, tag="sq", bufs=2, name="sq")
            nc.vector.tensor_mul(sq, xc, xc)
            psV = ps.tile([128, T], dt.float32, tag="V", bufs=vbufs, name="psv")
            for s in range(2):
                nc.tensor.matmul(psV[:, 512 * s:512 * s + 512], Wb("ones64"),
                                 sq[:, 512 * s:512 * s + 512])
            rstd = sb.tile([128, T], dt.float16, tag="rs", bufs=2, name="rstd")
            nc.scalar.activation(rstd, psV, AF.Abs_reciprocal_sqrt,
                                 bias=Wc("eps"), scale=1.0 / 64.0)
            nc.vector.tensor_mul(xn_out_ap, xc, rstd)

        def ln_wave(src, wtag):
            nonlocal_w = wno
            with tc.tile_pool(name=f"ln{wtag}s", bufs=2) as sb, \
                 tc.tile_pool(name=f"ln{wtag}p", bufs=2, space="PSUM") as ps:
                for k in fwd(nonlocal_w):
                    sl = slice(k * T, (k + 1) * T)
                    ln_chain(sb, ps, featA[:, sl] if src is featA else featB[:, sl],
                             xn_sb[:, sl])

        for i in range(2):
            # ---- LN1 (table: abs_reciprocal_sqrt) ----
            tc.no_sync_barrier()
            ln_wave(featA, f"a{i}")
            wno += 1
            # ---- attn + residual (no ACT funcs) ----
            tc.no_sync_barrier()
            with tc.tile_pool(name=f"at{i}p", bufs=4, space="PSUM") as ps:
                for k in fwd(wno):
                    sl = slice(k * T, (k + 1) * T)
                    psF2 = ps.tile([128, T], dt.float32, tag="F2")
                    for s in range(2):
                        nc.tensor.matmul(psF2[:, 512 * s:512 * s + 512], Wb(f"attn{i}"),
                                         xn_sb[:, k * T + 512 * s:k * T + 512 * s + 512])
                    nc.vector.scalar_tensor_tensor(
                        featB[:, sl], psF2, Wc(f"bvo{i}"), featA[:, sl],
                        ALU.add, ALU.add)
            wno += 1
            # ---- LN2 ----
            tc.no_sync_barrier()
            ln_wave(featB, f"c{i}")
            wno += 1
            # ---- MLP (ACT table: gelu) ----
            tc.no_sync_barrier()
            with tc.tile_pool(name=f"ml{i}s", bufs=6) as sb, \
                 tc.tile_pool(name=f"ml{i}p", bufs=2, space="PSUM") as ps:
                for k in fwd(wno):
                    sl = slice(k * T, (k + 1) * T)
                    gsb = []
                    for q in range(4):
                        gp = ps.tile([128, T], dt.float32, tag="G")
                        for s in range(2):
                            nc.tensor.matmul(gp[:, 512 * s:512 * s + 512],
                                             Wb(f"mlp1q{i}_{q}"),
                                             xn_sb[:, k * T + 512 * s:k * T + 512 * s + 512])
                        g = sb.tile([128, T], dt.float16, tag=f"g{q}")
                        nc.scalar.activation(g, gp, AF.Gelu, bias=Wc(f"b1gq{i}_{q}"))
                        gsb.append(g)
                    psF3 = ps.tile([128, T], dt.float32, tag="F3")
                    for q in range(4):
                        for s in range(2):
                            nc.tensor.matmul(psF3[:, 512 * s:512 * s + 512],
                                             Wb(f"mlp2q{i}_{q}"),
                                             gsb[q][:, 512 * s:512 * s + 512],
                                             start=(q == 0), stop=(q == 3))
                    nc.vector.scalar_tensor_tensor(
                        featA[:, sl], psF3, Wc(f"b2c{i}"), featB[:, sl],
                        ALU.add, ALU.add)
            wno += 1

        # ---- final LN + classifier (abs_reciprocal_sqrt) ----
        tc.no_sync_barrier()
        with tc.tile_pool(name="clss", bufs=2) as sb, \
             tc.tile_pool(name="clsp", bufs=1, space="PSUM") as ps:
            for k in fwd(wno):
                xn3 = sb.tile([128, T], dt.float16, tag="xn3", bufs=2)
                ln_chain(sb, ps, featA[:, k * T:(k + 1) * T], xn3, vbufs=1)
                psO = ps.tile([6, T], dt.float32, tag="O", bufs=1)
                for s in range(2):
                    nc.tensor.matmul(psO[:, 512 * s:512 * s + 512], Wb("cls", n=6),
                                     xn3[:, 512 * s:512 * s + 512])
                osb = sb.tile([6, T], dt.float32, tag="osb")
                nc.vector.tensor_scalar_add(osb, psO, Wc("bcls6", 0, 6))
                nc.sync.dma_start(y_d.ap()[:, k * T:(k + 1) * T], osb)
        wno += 1

    nc.compile()
    return nc


# ---------------------------------------------------------------- entry point
def _prep_x(w):
    """Host-side: build transposed bf16 input blobs xe [320,B], xo [48,B]."""
    eeg = w["eeg"].astype(F32)                       # [B, 62, 5]
    xeT = np.zeros((320, B_TOTAL), F32)
    xeT.reshape(5, 64, B_TOTAL)[:, 0:62, :] = eeg.transpose(2, 1, 0)
    xeT[62] = 1.0
    xoT = np.zeros((48, B_TOTAL), F32)
    xoT[0:33] = w["eog"].astype(F32)[:, 0, :].T
    xoT[33] = 1.0                                    # beta bias row
    return xeT.astype(F16), xoT.astype(F16)


def _make_in_maps(w):
    wbf, wf32, cols, colsb = _fold_weights(w)
    xeT, xoT = _prep_x(w)
    key = ("prog", wbf.shape[1], wf32.shape[1])
    in_maps = []
    for k in range(N_CORES):
        in_maps.append({
            "xe": np.ascontiguousarray(xeT[:, k * B_CORE:(k + 1) * B_CORE]),
            "xo": np.ascontiguousarray(xoT[:, k * B_CORE:(k + 1) * B_CORE]),
            "wbf": wbf, "wf32": wf32,
        })
    return key, in_maps, (wbf.shape[1], wf32.shape[1], cols, colsb)


def _unshard(res):
    out = np.empty((B_TOTAL, 3), F32)
    for k in range(N_CORES):
        y = res.results[k]["y_fm"].reshape(2, 3, N_TILE, T)
        out[k * B_CORE:(k + 1) * B_CORE] = (
            y.transpose(2, 0, 3, 1).reshape(B_CORE, 3))
    return out


def kernel(**inputs):
    w = {k: np.asarray(v) for k, v in inputs.items()}
    key, in_maps, (nbf, nf32, cols, colsb) = _make_in_maps(w)
    if key not in _CACHE:
        _CACHE[key] = _build(nbf, nf32, cols, colsb)
    nc = _CACHE[key]
    res = run_bass_kernel_spmd(nc, in_maps, core_ids=list(range(N_CORES)))
    return _unshard(res)


if __name__ == "__main__":
    import reference
    ins = {k: np.asarray(v) for k, v in reference.setup_inputs().items()}
    got = kernel(**ins)
    exp = np.asarray(reference.reference(**ins))
    err = np.abs(got - exp).max() / (np.abs(exp).max() + 1e-9)
    print("Relative error:", err)


# revision 39
# speedup vs baseline: 1.1896x; 1.0144x over previous
"""Trainium2 Bass kernel for nn_MCAF (dense_transformer).

Strategy: pure data-parallel over 8 NeuronCores (batch 131072 -> 16384/core).
v2: restructured for engine balance + PE p-state:
 - T=1024-col tiles (2 halves x 1024 batch elems = 2048 elems/tile, 8 tiles).
 - Wave structure grouped by ACT table set (exp | absrsqrt | gelu), zigzag
   tile order across waves so engines never drain at wave boundaries.
 - PSUM pools sized to exactly 8 banks per wave with >=2-deep rotation.
 - Front end: eog xo rows DMA'd straight into the y2 tile; h-conv and eog-raw
   contractions merged into one matmul chain; elu via min/exp with bf16 ops.
 - LN: center-matmul -> bf16 evac -> DVE square (4x) -> ones-matmul ->
   ACT Abs_reciprocal_sqrt -> DVE bf16 mul.
 - Classifier bias preloaded into PSUM via DMA; logits DMA'd PSUM->DRAM.
"""

import sys

sys.path.insert(0, "/opt/trn_rl_repo")

import numpy as np
import ml_dtypes

import concourse.bass as bass
import concourse.bacc as bacc
import concourse.tile as tile
from concourse import mybir
from concourse.bass_utils import run_bass_kernel_spmd

F16 = np.float16
F32 = np.float32

B_TOTAL = 131072
N_CORES = 8
B_CORE = B_TOTAL // N_CORES          # 16384
T = 1024                             # batch columns per half-tile
N_TILE = B_CORE // (2 * T)           # 8 tiles of 2 halves x T
AF = mybir.ActivationFunctionType
ALU = mybir.AluOpType
dt = mybir.dt


# ---------------------------------------------------------------- host folding
def _fold_weights(w):
    """Returns (wbf [128,NB] bf16 blob, wf32 [128,NF] f32 bias cols, maps)."""
    eeg_ow = w["eeg_ow"].astype(np.float64)
    wv = w["eeg_inw"][124:186].astype(np.float64)
    bv = w["eeg_inb"][124:186].astype(np.float64)
    Me5 = (eeg_ow @ wv) / 5.0                        # [62,62]
    c_e = eeg_ow @ bv + w["eeg_ob"].astype(np.float64)

    colsb = {}
    bblocks = []

    def addb(name, arr):
        colsb[name] = sum(b.shape[1] for b in bblocks)
        bblocks.append(arr)

    # obar blocks: x320 row g=64*l+c ; M=128 cols: [obar(62) 0 0 | obar(62) 0 0]
    for t in range(3):
        rows = 64 if t == 2 else 128
        blk = np.zeros((128, 128), np.float64)
        for r in range(rows):
            g = 128 * t + r
            c = g % 64
            if c < 62:
                blk[r, 0:62] = Me5[:, c]
                blk[r, 64:126] = Me5[:, c]
            elif g == 62:                            # host plants 1.0 in x320[:,62]
                blk[r, 0:62] = c_e
                blk[r, 64:126] = c_e
        addb(f"ob{t}", blk)

    # h blocks (eeg conv contraction): psHR rows {32h eeg-h} | raw rows later
    cw = w["eeg_cw"].astype(np.float64)              # [32,62,5]
    cwo = w["eog_cw"].astype(np.float64)             # [32,1,33]
    for t in range(3):
        rows = 64 if t == 2 else 128
        blk = np.zeros((128, 32), np.float64)
        for r in range(rows):
            g = 128 * t + r
            l, c = g // 64, g % 64
            if c < 62:
                blk[r, :] = cw[:, c, l]
        addb(f"h{t}", blk)
    # eog raw block (stored at partitions 64:112, read from y2[64:112])
    raw_blk = np.zeros((128, 32), np.float64)
    for l in range(33):
        raw_blk[64 + l, :] = cwo[:, 0, l]
    addb("ogr", raw_blk)

    # eog alpha block (stored at partitions 64:112)
    alpha33 = float(w["eog_inw"][2, 0]) * float(w["eog_ow"][0, 0]) / 33.0
    beta = float(w["eog_inb"][2]) * float(w["eog_ow"][0, 0]) + float(w["eog_ob"][0])
    alp_blk = np.zeros((128, 32), np.float64)
    alp_blk[64:64 + 33, :] = alpha33
    alp_blk[64 + 33, :] = beta                       # host plants 1.0 in xo row 33
    addb("alp", alp_blk)

    # fused (ef,of)->feat weights; psF rows [feat-h0 | feat-h1]
    fw = w["fus_w"].astype(np.float64)               # [64,128]
    W_e = fw[:, :64] @ w["eeg_fw"].astype(np.float64)    # [64,32]
    W_o = fw[:, 64:] @ w["eog_fw"].astype(np.float64)    # [64,32]
    elu_blk = np.zeros((128, 128), np.float64)
    elu_blk[0:32, 0:64] = W_e.T                      # eeg_h0 -> feat h0
    elu_blk[32:64, 64:128] = W_e.T                   # eeg_h1 -> feat h1
    elu_blk[64:96, 0:64] = W_o.T                     # og_h0  -> feat h0
    elu_blk[96:128, 64:128] = W_o.T                  # og_h1  -> feat h1
    addb("elu", elu_blk)

    def bdiag(blk):                                  # block-diag [128,128]
        out = np.zeros((128, 128))
        out[0:64, 0:64] = blk
        out[64:128, 64:128] = blk
        return out

    C = np.eye(64) - 1.0 / 64.0
    addb("center", bdiag(C))
    addb("ones64", bdiag(np.ones((64, 64))))

    pe0 = (np.arange(64) % 2).astype(np.float64)
    b_feat = (fw[:, :64] @ w["eeg_fb"].astype(np.float64)
              + fw[:, 64:] @ w["eog_fb"].astype(np.float64)
              + w["fus_b"].astype(np.float64) + pe0
              - W_e.sum(axis=1) - W_o.sum(axis=1))   # fold elu's (e'-1)

    lay = []
    for i in range(2):
        s1 = w["tl_ln1_s"][i].astype(np.float64)
        b1v = w["tl_ln1_b"][i].astype(np.float64)
        Wvo = w["tl_ow"][i].astype(np.float64) @ w["tl_inw"][i, 128:192].astype(np.float64)
        bvo = (w["tl_ow"][i].astype(np.float64) @ w["tl_inb"][i, 128:192].astype(np.float64)
               + w["tl_ob"][i].astype(np.float64))
        Wvo_s = Wvo * s1[None, :]
        bvo_t = Wvo @ b1v + bvo
        s2 = w["tl_ln2_s"][i].astype(np.float64)
        b2v = w["tl_ln2_b"][i].astype(np.float64)
        W1 = w["tl_w1"][i].astype(np.float64)        # [256,64]
        W1_s = W1 * s2[None, :]
        b1g = W1 @ b2v + w["tl_b1"][i].astype(np.float64)   # [256]
        W2 = w["tl_w2"][i].astype(np.float64)        # [64,256]
        b2c = w["tl_b2"][i].astype(np.float64)
        lay.append((Wvo_s, bvo_t, W1_s, b1g, W2, b2c))
        addb(f"attn{i}", bdiag(Wvo_s.T))
        m1 = W1_s.T                                  # [64,256]
        w2t = W2.T                                   # [256,64]
        for q in range(4):
            addb(f"mlp1q{i}_{q}", bdiag(m1[:, 64 * q:64 * q + 64]))
            addb(f"mlp2q{i}_{q}", bdiag(w2t[64 * q:64 * q + 64, :]))

    # fp8 DoubleRow mlp2 weights: per layer, chunk-pairs (q0,q1), (q2,q3)
    cols8 = {}
    f8blocks = []

    def add8(name, arr):
        cols8[name] = sum(b.shape[1] for b in f8blocks)
        f8blocks.append(arr)

    for i in range(2):
        w2t = lay[i][4].T                            # [256,64]
        for p in range(2):
            pair = np.concatenate(
                [bdiag(w2t[64 * (2 * p + j):64 * (2 * p + j) + 64, :])
                 for j in range(2)], axis=1)         # [128, 256]
            add8(f"mlp2d{i}_{p}", pair)
    wf8 = np.concatenate(f8blocks, axis=1)

    fn_s = w["fn_s"].astype(np.float64)
    fn_b = w["fn_b"].astype(np.float64)
    cls_w = w["cls_w"].astype(np.float64)
    cls_s = cls_w * fn_s[None, :]                    # [3,64]
    b_cls = cls_w @ fn_b + w["cls_b"].astype(np.float64)
    csT = cls_s.T                                    # [64,3]
    clsblk = np.zeros((128, 6))
    clsblk[0:64, 0:3] = csT
    clsblk[64:128, 3:6] = csT
    addb("cls", clsblk)

    wbf = np.concatenate(bblocks, axis=1)

    # --- f32 bias columns ---
    cols = {}
    blocks = []

    def add(name, arr):
        cols[name] = sum(b.shape[1] for b in blocks)
        blocks.append(arr)

    def col(vals128):
        return np.asarray(vals128, np.float64).reshape(128, 1)

    cb_e = w["eeg_cb"].astype(np.float64)
    cb_o = w["eog_cb"].astype(np.float64)
    add("cbcol", col(np.concatenate([cb_e, cb_e, cb_o, cb_o])))
    add("bfeat", col(np.concatenate([b_feat, b_feat])))
    for i in range(2):
        add(f"bvo{i}", col(np.concatenate([lay[i][1], lay[i][1]])))
        for q in range(4):
            add(f"b1gq{i}_{q}", col(np.concatenate([lay[i][3][64 * q:64 * q + 64]] * 2)))
        add(f"b2c{i}", col(np.concatenate([lay[i][5], lay[i][5]])))
    add("eps", col(np.full(128, 1e-5)))
    add("zero", col(np.zeros(128)))
    bc6 = np.zeros(128)
    bc6[0:3] = b_cls
    bc6[3:6] = b_cls
    add("bcls6", col(bc6))

    wf32 = np.concatenate(blocks, axis=1)
    return (wbf.astype(F16), wf32.astype(F32),
            wf8.astype(ml_dtypes.float8_e4m3fn), cols, colsb, cols8)


# ---------------------------------------------------------------- device build
_CACHE = {}


def _build(nbf, nf32, nf8, cols, colsb, cols8):
    nc = bacc.Bacc("TRN2", target_bir_lowering=False, debug=False)
    xe_d = nc.dram_tensor("xe", [320, B_CORE], dt.float16, kind="ExternalInput")
    xo_d = nc.dram_tensor("xo", [48, B_CORE], dt.float16, kind="ExternalInput")
    wbf_d = nc.dram_tensor("wbf", [128, nbf], dt.float16, kind="ExternalInput")
    wf_d = nc.dram_tensor("wf32", [128, nf32], dt.float32, kind="ExternalInput")
    wf8_d = nc.dram_tensor("wf8", [128, nf8], dt.float8e4, kind="ExternalInput")
    y_d = nc.dram_tensor("y_fm", [6, N_TILE * T], dt.float32, kind="ExternalOutput")

    # persistent sbuf arrays
    wbf_sb = nc.alloc_sbuf_tensor("wbf_sb", [128, nbf], dt.float16).ap()
    wf_sb = nc.alloc_sbuf_tensor("wf_sb", [128, nf32], dt.float32).ap()
    wf8_sb = nc.alloc_sbuf_tensor("wf8_sb", [128, nf8], dt.float8e4).ap()
    featA = nc.alloc_sbuf_tensor("featA", [128, N_TILE * T], dt.float16).ap()
    featB = nc.alloc_sbuf_tensor("featB", [128, N_TILE * T], dt.float16).ap()
    xn_sb = nc.alloc_sbuf_tensor("xn_sb", [128, N_TILE * T], dt.float16).ap()

    def Wb(name, r0=0, r1=128, n=128):
        c0 = colsb[name]
        return wbf_sb[r0:r1, c0:c0 + n]

    def W8(name):                                    # fp8 DR pair [128,2,128]
        c0 = cols8[name]
        return wf8_sb[:, c0:c0 + 256].rearrange("p (two m) -> p two m", two=2)

    def Wc(name, r0=0, r1=128):                      # bias columns
        return wf_sb[r0:r1, cols[name]:cols[name] + 1]

    def fwd(w):                                      # zigzag tile order per wave
        return range(N_TILE) if w % 2 == 0 else range(N_TILE - 1, -1, -1)

    wno = 0

    with tile.TileContext(nc) as tc:
        nc.sync.dma_start(wbf_sb, wbf_d.ap())
        nc.sync.dma_start(wf_sb, wf_d.ap())
        nc.sync.dma_start(wf8_sb, wf8_d.ap())

        # =================== WAVE 1: front end (ACT table: exp) ===============
        with tc.tile_pool(name="w1x", bufs=2) as xp, \
             tc.tile_pool(name="w1s", bufs=2) as sb, \
             tc.tile_pool(name="w1p", bufs=1, space="PSUM") as ps:
            for k in fwd(wno):
                c0 = k * 2 * T                       # first batch col of tile
                xt = xp.tile([128, 3 * 2 * T], dt.float16, tag="xt")
                nc.sync.dma_start(xt[:, 0:2 * T], xe_d.ap()[0:128, c0:c0 + 2 * T])
                nc.sync.dma_start(xt[:, 2 * T:4 * T], xe_d.ap()[128:256, c0:c0 + 2 * T])
                nc.sync.dma_start(xt[0:64, 4 * T:6 * T], xe_d.ap()[256:320, c0:c0 + 2 * T])
                y2s = []
                for h in range(2):
                    y2 = xp.tile([112, T], dt.float16, tag=f"y2{h}")
                    nc.sync.dma_start(y2[64:112], xo_d.ap()[:, c0 + h * T:c0 + (h + 1) * T])
                    y2s.append(y2)

                psHR = ps.tile([128, T], dt.float32, tag="HR", bufs=1)
                psA = ps.tile([128, T], dt.float32, tag="A")
                for h in range(2):
                    ob2 = sb.tile([128, T], dt.float16, tag=f"ob2{h}")
                    # obar: 3 chunks x 2 slices, chunk-major for weight reuse
                    psobs = [ps.tile([128, 512], dt.float32, tag="OB", bufs=2,
                                     name=f"psob{h}{j}") for j in range(2)]
                    for t in range(3):
                        kk = 64 if t == 2 else 128
                        for s in range(2):
                            mv = xt[0:kk, 2 * T * t + h * T + 512 * s:
                                    2 * T * t + h * T + 512 * s + 512]
                            nc.tensor.matmul(psobs[s], Wb(f"ob{t}")[0:kk], mv,
                                             start=(t == 0), stop=(t == 2))
                    for s in range(2):
                        nc.scalar.activation(ob2[:, 512 * s:512 * s + 512], psobs[s],
                                             AF.Identity, bias=Wc("zero"))
                    # y = x * obar  (bf16, 4x DVE)
                    y0 = sb.tile([128, T], dt.float16, tag=f"y0{h}")
                    y1 = sb.tile([128, T], dt.float16, tag=f"y1{h}")
                    nc.vector.tensor_mul(y0, xt[:, h * T:(h + 1) * T], ob2)
                    nc.vector.tensor_mul(y1, xt[:, 2 * T + h * T:2 * T + (h + 1) * T], ob2)
                    nc.vector.tensor_mul(y2s[h][0:64],
                                         xt[0:64, 4 * T + h * T:4 * T + (h + 1) * T],
                                         ob2[0:64])
                    # eeg h chain -> psHR rows 32h:32h+32
                    for t in range(3):
                        kk = 64 if t == 2 else 128
                        for s in range(2):
                            if t == 2:
                                mv = y2s[h][0:64, 512 * s:512 * s + 512]
                            else:
                                yy = y0 if t == 0 else y1
                                mv = yy[:, 512 * s:512 * s + 512]
                            nc.tensor.matmul(psHR[32 * h:32 * h + 32,
                                                  512 * s:512 * s + 512],
                                             Wb(f"h{t}", 0, kk, n=32), mv,
                                             start=(t == 0), stop=(t == 2))
                # eog alpha / raw MMs (psR reuses the OB psum ring)
                psRs = [ps.tile([128, 512], dt.float32, tag="OB", bufs=2,
                                name=f"psr{j}") for j in range(2)]
                for h in range(2):
                    o2 = 64 + 32 * h
                    for s in range(2):
                        nc.tensor.matmul(psA[o2:o2 + 32, 512 * s:512 * s + 512],
                                         Wb("alp", 64, 112, n=32),
                                         y2s[h][64:112, 512 * s:512 * s + 512],
                                         tile_position=(64, o2))
                        nc.tensor.matmul(psRs[s][o2:o2 + 32],
                                         Wb("ogr", 64, 112, n=32),
                                         y2s[h][64:112, 512 * s:512 * s + 512],
                                         tile_position=(64, o2))
                # og = raw * (alpha*mean + beta): evac alpha, mult into psHR
                o2a = sb.tile([128, T], dt.float16, tag="o2a")
                nc.scalar.activation(o2a[64:128], psA[64:128], AF.Identity,
                                     bias=Wc("zero", 64, 128))
                for s in range(2):
                    nc.vector.tensor_mul(psHR[64:128, 512 * s:512 * s + 512],
                                         psRs[s][64:128],
                                         o2a[64:128, 512 * s:512 * s + 512])
                # elu(z)+1 = max(z,0) + exp(min(z,0)); -1 folded into bfeat
                r1 = sb.tile([128, T], dt.float16, tag="r1")
                sm = sb.tile([128, T], dt.float16, tag="sm")
                e1 = sb.tile([128, T], dt.float16, tag="e1")
                eluT = sb.tile([128, T], dt.float16, tag="elu")
                nc.scalar.activation(r1, psHR, AF.Relu, bias=Wc("cbcol"))
                nc.vector.tensor_scalar(sm, psHR, Wc("cbcol"), 0.0, ALU.add, ALU.min)
                nc.scalar.activation(e1, sm, AF.Exp, bias=Wc("zero"))
                nc.vector.tensor_add(eluT, r1, e1)
                # feat = W_elu.T @ elu (+b_feat)
                psF = ps.tile([128, T], dt.float32, tag="F", bufs=1)
                for s in range(2):
                    nc.tensor.matmul(psF[:, 512 * s:512 * s + 512], Wb("elu"),
                                     eluT[:, 512 * s:512 * s + 512])
                nc.scalar.activation(featA[:, c0 // 2:c0 // 2 + T],
                                     psF, AF.Identity, bias=Wc("bfeat"))
        wno += 1

        # =================== transformer ===================
        def ln_chain(sb, ps, src_ap, xn_out_ap, vbufs=2):
            """LN on one [128,T] tile: ACT evac -> fp16 DVE sq -> ones-MM ->
            AbsRsqrt -> fp16 DVE mul. src_ap is fp16 SBUF [128, T]."""
            psXC = ps.tile([128, T], dt.float32, tag="XC", bufs=2, name="psxc")
            for s in range(2):
                nc.tensor.matmul(psXC[:, 512 * s:512 * s + 512], Wb("center"),
                                 src_ap[:, 512 * s:512 * s + 512])
            xc = sb.tile([128, T], dt.float16, tag="xc", bufs=2, name="xc")
            nc.scalar.activation(xc, psXC, AF.Identity, bias=Wc("zero"))
            sq = sb.tile([128, T], dt.float16, tag="sq", bufs=2, name="sq")
            nc.vector.tensor_mul(sq, xc, xc)
            psV = ps.tile([128, T], dt.float32, tag="V", bufs=vbufs, name="psv")
            for s in range(2):
                nc.tensor.matmul(psV[:, 512 * s:512 * s + 512], Wb("ones64"),
                                 sq[:, 512 * s:512 * s + 512])
            rstd = sb.tile([128, T], dt.float16, tag="rs", bufs=2, name="rstd")
            nc.scalar.activation(rstd, psV, AF.Abs_reciprocal_sqrt,
                                 bias=Wc("eps"), scale=1.0 / 64.0)
            nc.vector.tensor_mul(xn_out_ap, xc, rstd)

        def ln_wave(src, wtag):
            nonlocal_w = wno
            with tc.tile_pool(name=f"ln{wtag}s", bufs=2) as sb, \
                 tc.tile_pool(name=f"ln{wtag}p", bufs=2, space="PSUM") as ps:
                for k in fwd(nonlocal_w):
                    sl = slice(k * T, (k + 1) * T)
                    ln_chain(sb, ps, featA[:, sl] if src is featA else featB[:, sl],
                             xn_sb[:, sl])

        for i in range(2):
            # ---- LN1 (table: abs_reciprocal_sqrt) ----
            tc.no_sync_barrier()
            ln_wave(featA, f"a{i}")
            wno += 1
            # ---- attn + residual (no ACT funcs) ----
            tc.no_sync_barrier()
            with tc.tile_pool(name=f"at{i}p", bufs=4, space="PSUM") as ps:
                for k in fwd(wno):
                    sl = slice(k * T, (k + 1) * T)
                    psF2 = ps.tile([128, T], dt.float32, tag="F2")
                    for s in range(2):
                        nc.tensor.matmul(psF2[:, 512 * s:512 * s + 512], Wb(f"attn{i}"),
                                         xn_sb[:, k * T + 512 * s:k * T + 512 * s + 512])
                    nc.vector.scalar_tensor_tensor(
                        featB[:, sl], psF2, Wc(f"bvo{i}"), featA[:, sl],
                        ALU.add, ALU.add)
            wno += 1
            # ---- LN2 ----
            tc.no_sync_barrier()
            ln_wave(featB, f"c{i}")
            wno += 1
            # ---- MLP (ACT table: gelu) ----
            tc.no_sync_barrier()
            with tc.tile_pool(name=f"ml{i}s", bufs=6) as sb, \
                 tc.tile_pool(name=f"ml{i}p", bufs=2, space="PSUM") as ps:
                for k in fwd(wno):
                    sl = slice(k * T, (k + 1) * T)
                    gsb = []
                    for q in range(4):
                        gp = ps.tile([128, T], dt.float32, tag="G", name=f"gp{q}")
                        for s in range(2):
                            nc.tensor.matmul(gp[:, 512 * s:512 * s + 512],
                                             Wb(f"mlp1q{i}_{q}"),
                                             xn_sb[:, k * T + 512 * s:k * T + 512 * s + 512])
                        g = sb.tile([128, T], dt.float16, tag=f"g{q}")
                        nc.scalar.activation(g, gp, AF.Gelu, bias=Wc(f"b1gq{i}_{q}"))
                        gsb.append(g)
                    psF3 = ps.tile([128, T], dt.float32, tag="F3")
                    for q in range(4):
                        for s in range(2):
                            nc.tensor.matmul(psF3[:, 512 * s:512 * s + 512],
                                             Wb(f"mlp2q{i}_{q}"),
                                             gsb[q][:, 512 * s:512 * s + 512],
                                             start=(q == 0), stop=(q == 3))
                    nc.vector.scalar_tensor_tensor(
                        featA[:, sl], psF3, Wc(f"b2c{i}"), featB[:, sl],
                        ALU.add, ALU.add)
            wno += 1

        # ---- final LN + classifier (abs_reciprocal_sqrt) ----
        tc.no_sync_barrier()
        with tc.tile_pool(name="clss", bufs=2) as sb, \
             tc.tile_pool(name="clsp", bufs=1, space="PSUM") as ps:
            for k in fwd(wno):
                xn3 = sb.tile([128, T], dt.float16, tag="xn3", bufs=2)
                ln_chain(sb, ps, featA[:, k * T:(k + 1) * T], xn3, vbufs=1)
                psO = ps.tile([6, T], dt.float32, tag="O", bufs=1)
                for s in range(2):
                    nc.tensor.matmul(psO[:, 512 * s:512 * s + 512], Wb("cls", n=6),
                                     xn3[:, 512 * s:512 * s + 512])
                osb = sb.tile([6, T], dt.float32, tag="osb")
                nc.vector.tensor_scalar_add(osb, psO, Wc("bcls6", 0, 6))
                nc.sync.dma_start(y_d.ap()[:, k * T:(k + 1) * T], osb)
        wno += 1

    nc.compile()
    return nc


# ---------------------------------------------------------------- entry point
def _prep_x(w):
    """Host-side: build transposed bf16 input blobs xe [320,B], xo [48,B]."""
    eeg = w["eeg"].astype(F32)                       # [B, 62, 5]
    xeT = np.zeros((320, B_TOTAL), F32)
    xeT.reshape(5, 64, B_TOTAL)[:, 0:62, :] = eeg.transpose(2, 1, 0)
    xeT[62] = 1.0
    xoT = np.zeros((48, B_TOTAL), F32)
    xoT[0:33] = w["eog"].astype(F32)[:, 0, :].T
    xoT[33] = 1.0                                    # beta bias row
    return xeT.astype(F16), xoT.astype(F16)


def _make_in_maps(w):
    wbf, wf32, wf8, cols, colsb, cols8 = _fold_weights(w)
    xeT, xoT = _prep_x(w)
    key = ("prog", wbf.shape[1], wf32.shape[1], wf8.shape[1])
    in_maps = []
    for k in range(N_CORES):
        in_maps.append({
            "xe": np.ascontiguousarray(xeT[:, k * B_CORE:(k + 1) * B_CORE]),
            "xo": np.ascontiguousarray(xoT[:, k * B_CORE:(k + 1) * B_CORE]),
            "wbf": wbf, "wf32": wf32, "wf8": wf8,
        })
    return key, in_maps, (wbf.shape[1], wf32.shape[1], wf8.shape[1], cols, colsb, cols8)


def _unshard(res):
    out = np.empty((B_TOTAL, 3), F32)
    for k in range(N_CORES):
        y = res.results[k]["y_fm"].reshape(2, 3, N_TILE, T)
        out[k * B_CORE:(k + 1) * B_CORE] = (
            y.transpose(2, 0, 3, 1).reshape(B_CORE, 3))
    return out


def kernel(**inputs):
    w = {k: np.asarray(v) for k, v in inputs.items()}
    key, in_maps, bargs = _make_in_maps(w)
    if key not in _CACHE:
        _CACHE[key] = _build(*bargs)
    nc = _CACHE[key]
    res = run_bass_kernel_spmd(nc, in_maps, core_ids=list(range(N_CORES)))
    return _unshard(res)


if __name__ == "__main__":
    import reference
    ins = {k: np.asarray(v) for k, v in reference.setup_inputs().items()}
    got = kernel(**ins)
    exp = np.asarray(reference.reference(**ins))
    err = np.abs(got - exp).max() / (np.abs(exp).max() + 1e-9)
    print("Relative error:", err)


# revision 40
# speedup vs baseline: 1.1981x; 1.0072x over previous
"""Trainium2 Bass kernel for nn_MCAF (dense_transformer).

Strategy: pure data-parallel over 8 NeuronCores (batch 131072 -> 16384/core).
v2: restructured for engine balance + PE p-state:
 - T=1024-col tiles (2 halves x 1024 batch elems = 2048 elems/tile, 8 tiles).
 - Wave structure grouped by ACT table set (exp | absrsqrt | gelu), zigzag
   tile order across waves so engines never drain at wave boundaries.
 - PSUM pools sized to exactly 8 banks per wave with >=2-deep rotation.
 - Front end: eog xo rows DMA'd straight into the y2 tile; h-conv and eog-raw
   contractions merged into one matmul chain; elu via min/exp with bf16 ops.
 - LN: center-matmul -> bf16 evac -> DVE square (4x) -> ones-matmul ->
   ACT Abs_reciprocal_sqrt -> DVE bf16 mul.
 - Classifier bias preloaded into PSUM via DMA; logits DMA'd PSUM->DRAM.
"""

import sys

sys.path.insert(0, "/opt/trn_rl_repo")

import numpy as np
import ml_dtypes

import concourse.bass as bass
import concourse.bacc as bacc
import concourse.tile as tile
from concourse import mybir
from concourse.bass_utils import run_bass_kernel_spmd

F16 = np.float16
F32 = np.float32

B_TOTAL = 131072
N_CORES = 8
B_CORE = B_TOTAL // N_CORES          # 16384
T = 1024                             # batch columns per half-tile
N_TILE = B_CORE // (2 * T)           # 8 tiles of 2 halves x T
AF = mybir.ActivationFunctionType
ALU = mybir.AluOpType
dt = mybir.dt


# ---------------------------------------------------------------- host folding
def _fold_weights(w):
    """Returns (wbf [128,NB] bf16 blob, wf32 [128,NF] f32 bias cols, maps)."""
    eeg_ow = w["eeg_ow"].astype(np.float64)
    wv = w["eeg_inw"][124:186].astype(np.float64)
    bv = w["eeg_inb"][124:186].astype(np.float64)
    Me5 = (eeg_ow @ wv) / 5.0                        # [62,62]
    c_e = eeg_ow @ bv + w["eeg_ob"].astype(np.float64)

    colsb = {}
    bblocks = []

    def addb(name, arr):
        colsb[name] = sum(b.shape[1] for b in bblocks)
        bblocks.append(arr)

    # obar blocks: x320 row g=64*l+c ; M=128 cols: [obar(62) 0 0 | obar(62) 0 0]
    for t in range(3):
        rows = 64 if t == 2 else 128
        blk = np.zeros((128, 128), np.float64)
        for r in range(rows):
            g = 128 * t + r
            c = g % 64
            if c < 62:
                blk[r, 0:62] = Me5[:, c]
                blk[r, 64:126] = Me5[:, c]
            elif g == 62:                            # host plants 1.0 in x320[:,62]
                blk[r, 0:62] = c_e
                blk[r, 64:126] = c_e
        addb(f"ob{t}", blk)

    # h blocks (eeg conv contraction): psHR rows {32h eeg-h} | raw rows later
    cw = w["eeg_cw"].astype(np.float64)              # [32,62,5]
    cwo = w["eog_cw"].astype(np.float64)             # [32,1,33]
    for t in range(3):
        rows = 64 if t == 2 else 128
        blk = np.zeros((128, 32), np.float64)
        for r in range(rows):
            g = 128 * t + r
            l, c = g // 64, g % 64
            if c < 62:
                blk[r, :] = cw[:, c, l]
        addb(f"h{t}", blk)
    # eog raw block (stored at partitions 64:112, read from y2[64:112])
    raw_blk = np.zeros((128, 32), np.float64)
    for l in range(33):
        raw_blk[64 + l, :] = cwo[:, 0, l]
    addb("ogr", raw_blk)

    # eog alpha block (stored at partitions 64:112)
    alpha33 = float(w["eog_inw"][2, 0]) * float(w["eog_ow"][0, 0]) / 33.0
    beta = float(w["eog_inb"][2]) * float(w["eog_ow"][0, 0]) + float(w["eog_ob"][0])
    alp_blk = np.zeros((128, 32), np.float64)
    alp_blk[64:64 + 33, :] = alpha33
    alp_blk[64 + 33, :] = beta                       # host plants 1.0 in xo row 33
    addb("alp", alp_blk)

    # fused (ef,of)->feat weights; psF rows [feat-h0 | feat-h1]
    fw = w["fus_w"].astype(np.float64)               # [64,128]
    W_e = fw[:, :64] @ w["eeg_fw"].astype(np.float64)    # [64,32]
    W_o = fw[:, 64:] @ w["eog_fw"].astype(np.float64)    # [64,32]
    elu_blk = np.zeros((128, 128), np.float64)
    elu_blk[0:32, 0:64] = W_e.T                      # eeg_h0 -> feat h0
    elu_blk[32:64, 64:128] = W_e.T                   # eeg_h1 -> feat h1
    elu_blk[64:96, 0:64] = W_o.T                     # og_h0  -> feat h0
    elu_blk[96:128, 64:128] = W_o.T                  # og_h1  -> feat h1
    addb("elu", elu_blk)

    def bdiag(blk):                                  # block-diag [128,128]
        out = np.zeros((128, 128))
        out[0:64, 0:64] = blk
        out[64:128, 64:128] = blk
        return out

    C = np.eye(64) - 1.0 / 64.0
    addb("center", bdiag(C))
    addb("ones64", bdiag(np.ones((64, 64))))

    pe0 = (np.arange(64) % 2).astype(np.float64)
    b_feat = (fw[:, :64] @ w["eeg_fb"].astype(np.float64)
              + fw[:, 64:] @ w["eog_fb"].astype(np.float64)
              + w["fus_b"].astype(np.float64) + pe0
              - W_e.sum(axis=1) - W_o.sum(axis=1))   # fold elu's (e'-1)

    lay = []
    for i in range(2):
        s1 = w["tl_ln1_s"][i].astype(np.float64)
        b1v = w["tl_ln1_b"][i].astype(np.float64)
        Wvo = w["tl_ow"][i].astype(np.float64) @ w["tl_inw"][i, 128:192].astype(np.float64)
        bvo = (w["tl_ow"][i].astype(np.float64) @ w["tl_inb"][i, 128:192].astype(np.float64)
               + w["tl_ob"][i].astype(np.float64))
        Wvo_s = Wvo * s1[None, :]
        bvo_t = Wvo @ b1v + bvo
        s2 = w["tl_ln2_s"][i].astype(np.float64)
        b2v = w["tl_ln2_b"][i].astype(np.float64)
        W1 = w["tl_w1"][i].astype(np.float64)        # [256,64]
        W1_s = W1 * s2[None, :]
        b1g = W1 @ b2v + w["tl_b1"][i].astype(np.float64)   # [256]
        W2 = w["tl_w2"][i].astype(np.float64)        # [64,256]
        b2c = w["tl_b2"][i].astype(np.float64)
        lay.append((Wvo_s, bvo_t, W1_s, b1g, W2, b2c))
        addb(f"attn{i}", bdiag(Wvo_s.T))
        m1 = W1_s.T                                  # [64,256]
        w2t = W2.T                                   # [256,64]
        for q in range(4):
            addb(f"mlp1q{i}_{q}", bdiag(m1[:, 64 * q:64 * q + 64]))
            addb(f"mlp2q{i}_{q}", bdiag(w2t[64 * q:64 * q + 64, :]))

    # fp8 DoubleRow mlp2 weights: per layer, chunk-pairs (q0,q1), (q2,q3)
    cols8 = {}
    f8blocks = []

    def add8(name, arr):
        cols8[name] = sum(b.shape[1] for b in f8blocks)
        f8blocks.append(arr)

    for i in range(2):
        w2t = lay[i][4].T                            # [256,64]
        for p in range(2):
            pair = np.concatenate(
                [bdiag(w2t[64 * (2 * p + j):64 * (2 * p + j) + 64, :])
                 for j in range(2)], axis=1)         # [128, 256]
            add8(f"mlp2d{i}_{p}", pair)
    wf8 = np.concatenate(f8blocks, axis=1)

    fn_s = w["fn_s"].astype(np.float64)
    fn_b = w["fn_b"].astype(np.float64)
    cls_w = w["cls_w"].astype(np.float64)
    cls_s = cls_w * fn_s[None, :]                    # [3,64]
    b_cls = cls_w @ fn_b + w["cls_b"].astype(np.float64)
    csT = cls_s.T                                    # [64,3]
    clsblk = np.zeros((128, 6))
    clsblk[0:64, 0:3] = csT
    clsblk[64:128, 3:6] = csT
    addb("cls", clsblk)

    wbf = np.concatenate(bblocks, axis=1)

    # --- f32 bias columns ---
    cols = {}
    blocks = []

    def add(name, arr):
        cols[name] = sum(b.shape[1] for b in blocks)
        blocks.append(arr)

    def col(vals128):
        return np.asarray(vals128, np.float64).reshape(128, 1)

    cb_e = w["eeg_cb"].astype(np.float64)
    cb_o = w["eog_cb"].astype(np.float64)
    add("cbcol", col(np.concatenate([cb_e, cb_e, cb_o, cb_o])))
    add("bfeat", col(np.concatenate([b_feat, b_feat])))
    for i in range(2):
        add(f"bvo{i}", col(np.concatenate([lay[i][1], lay[i][1]])))
        for q in range(4):
            add(f"b1gq{i}_{q}", col(np.concatenate([lay[i][3][64 * q:64 * q + 64]] * 2)))
        add(f"b2c{i}", col(np.concatenate([lay[i][5], lay[i][5]])))
    add("eps", col(np.full(128, 1e-5)))
    add("zero", col(np.zeros(128)))
    bc6 = np.zeros(128)
    bc6[0:3] = b_cls
    bc6[3:6] = b_cls
    add("bcls6", col(bc6))

    wf32 = np.concatenate(blocks, axis=1)
    return (wbf.astype(F16), wf32.astype(F32),
            wf8.astype(ml_dtypes.float8_e4m3fn), cols, colsb, cols8)


# ---------------------------------------------------------------- device build
_CACHE = {}


def _build(nbf, nf32, nf8, cols, colsb, cols8):
    nc = bacc.Bacc("TRN2", target_bir_lowering=False, debug=False)
    xe_d = nc.dram_tensor("xe", [320, B_CORE], dt.float16, kind="ExternalInput")
    xo_d = nc.dram_tensor("xo", [48, B_CORE], dt.float16, kind="ExternalInput")
    wbf_d = nc.dram_tensor("wbf", [128, nbf], dt.float16, kind="ExternalInput")
    wf_d = nc.dram_tensor("wf32", [128, nf32], dt.float32, kind="ExternalInput")
    wf8_d = nc.dram_tensor("wf8", [128, nf8], dt.float8e4, kind="ExternalInput")
    y_d = nc.dram_tensor("y_fm", [6, N_TILE * T], dt.float32, kind="ExternalOutput")

    # persistent sbuf arrays
    wbf_sb = nc.alloc_sbuf_tensor("wbf_sb", [128, nbf], dt.float16).ap()
    wf_sb = nc.alloc_sbuf_tensor("wf_sb", [128, nf32], dt.float32).ap()
    wf8_sb = nc.alloc_sbuf_tensor("wf8_sb", [128, nf8], dt.float8e4).ap()
    featA = nc.alloc_sbuf_tensor("featA", [128, N_TILE * T], dt.float16).ap()
    featB = nc.alloc_sbuf_tensor("featB", [128, N_TILE * T], dt.float16).ap()
    xn_sb = nc.alloc_sbuf_tensor("xn_sb", [128, N_TILE * T], dt.float16).ap()

    def Wb(name, r0=0, r1=128, n=128):
        c0 = colsb[name]
        return wbf_sb[r0:r1, c0:c0 + n]

    def W8(name):                                    # fp8 DR pair [128,2,128]
        c0 = cols8[name]
        return wf8_sb[:, c0:c0 + 256].rearrange("p (two m) -> p two m", two=2)

    def Wc(name, r0=0, r1=128):                      # bias columns
        return wf_sb[r0:r1, cols[name]:cols[name] + 1]

    def fwd(w):                                      # same order every wave:
        return range(N_TILE)                         # waves pipeline per-tile

    wno = 0

    with tile.TileContext(nc) as tc:
        nc.sync.dma_start(wbf_sb, wbf_d.ap())
        nc.sync.dma_start(wf_sb, wf_d.ap())
        nc.sync.dma_start(wf8_sb, wf8_d.ap())

        # =================== WAVE 1: front end (ACT table: exp) ===============
        with tc.tile_pool(name="w1x", bufs=2) as xp, \
             tc.tile_pool(name="w1s", bufs=2) as sb, \
             tc.tile_pool(name="w1p", bufs=1, space="PSUM") as ps:
            for k in fwd(wno):
                c0 = k * 2 * T                       # first batch col of tile
                xt = xp.tile([128, 3 * 2 * T], dt.float16, tag="xt")
                nc.sync.dma_start(xt[:, 0:2 * T], xe_d.ap()[0:128, c0:c0 + 2 * T])
                nc.sync.dma_start(xt[:, 2 * T:4 * T], xe_d.ap()[128:256, c0:c0 + 2 * T])
                nc.sync.dma_start(xt[0:64, 4 * T:6 * T], xe_d.ap()[256:320, c0:c0 + 2 * T])
                y2s = []
                for h in range(2):
                    y2 = xp.tile([112, T], dt.float16, tag=f"y2{h}")
                    nc.sync.dma_start(y2[64:112], xo_d.ap()[:, c0 + h * T:c0 + (h + 1) * T])
                    y2s.append(y2)

                psHR = ps.tile([128, T], dt.float32, tag="HR", bufs=1)
                psA = ps.tile([128, T], dt.float32, tag="A")
                for h in range(2):
                    ob2 = sb.tile([128, T], dt.float16, tag=f"ob2{h}")
                    # obar: 3 chunks x 2 slices, chunk-major for weight reuse
                    psobs = [ps.tile([128, 512], dt.float32, tag="OB", bufs=2,
                                     name=f"psob{h}{j}") for j in range(2)]
                    for t in range(3):
                        kk = 64 if t == 2 else 128
                        for s in range(2):
                            mv = xt[0:kk, 2 * T * t + h * T + 512 * s:
                                    2 * T * t + h * T + 512 * s + 512]
                            nc.tensor.matmul(psobs[s], Wb(f"ob{t}")[0:kk], mv,
                                             start=(t == 0), stop=(t == 2))
                    for s in range(2):
                        nc.scalar.activation(ob2[:, 512 * s:512 * s + 512], psobs[s],
                                             AF.Identity, bias=Wc("zero"))
                    # y = x * obar  (bf16, 4x DVE)
                    y0 = sb.tile([128, T], dt.float16, tag=f"y0{h}")
                    y1 = sb.tile([128, T], dt.float16, tag=f"y1{h}")
                    nc.vector.tensor_mul(y0, xt[:, h * T:(h + 1) * T], ob2)
                    nc.vector.tensor_mul(y1, xt[:, 2 * T + h * T:2 * T + (h + 1) * T], ob2)
                    nc.vector.tensor_mul(y2s[h][0:64],
                                         xt[0:64, 4 * T + h * T:4 * T + (h + 1) * T],
                                         ob2[0:64])
                    # eeg h chain -> psHR rows 32h:32h+32
                    for t in range(3):
                        kk = 64 if t == 2 else 128
                        for s in range(2):
                            if t == 2:
                                mv = y2s[h][0:64, 512 * s:512 * s + 512]
                            else:
                                yy = y0 if t == 0 else y1
                                mv = yy[:, 512 * s:512 * s + 512]
                            nc.tensor.matmul(psHR[32 * h:32 * h + 32,
                                                  512 * s:512 * s + 512],
                                             Wb(f"h{t}", 0, kk, n=32), mv,
                                             start=(t == 0), stop=(t == 2))
                # eog alpha / raw MMs (psR reuses the OB psum ring)
                psRs = [ps.tile([128, 512], dt.float32, tag="OB", bufs=2,
                                name=f"psr{j}") for j in range(2)]
                for h in range(2):
                    o2 = 64 + 32 * h
                    for s in range(2):
                        nc.tensor.matmul(psA[o2:o2 + 32, 512 * s:512 * s + 512],
                                         Wb("alp", 64, 112, n=32),
                                         y2s[h][64:112, 512 * s:512 * s + 512],
                                         tile_position=(64, o2))
                        nc.tensor.matmul(psRs[s][o2:o2 + 32],
                                         Wb("ogr", 64, 112, n=32),
                                         y2s[h][64:112, 512 * s:512 * s + 512],
                                         tile_position=(64, o2))
                # og = raw * (alpha*mean + beta): evac alpha, mult into psHR
                o2a = sb.tile([128, T], dt.float16, tag="o2a")
                nc.scalar.activation(o2a[64:128], psA[64:128], AF.Identity,
                                     bias=Wc("zero", 64, 128))
                for s in range(2):
                    nc.vector.tensor_mul(psHR[64:128, 512 * s:512 * s + 512],
                                         psRs[s][64:128],
                                         o2a[64:128, 512 * s:512 * s + 512])
                # elu(z)+1 = max(z,0) + exp(min(z,0)); -1 folded into bfeat
                r1 = sb.tile([128, T], dt.float16, tag="r1")
                sm = sb.tile([128, T], dt.float16, tag="sm")
                e1 = sb.tile([128, T], dt.float16, tag="e1")
                eluT = sb.tile([128, T], dt.float16, tag="elu")
                nc.scalar.activation(r1, psHR, AF.Relu, bias=Wc("cbcol"))
                nc.vector.tensor_scalar(sm, psHR, Wc("cbcol"), 0.0, ALU.add, ALU.min)
                nc.scalar.activation(e1, sm, AF.Exp, bias=Wc("zero"))
                nc.vector.tensor_add(eluT, r1, e1)
                # feat = W_elu.T @ elu (+b_feat)
                psF = ps.tile([128, T], dt.float32, tag="F", bufs=1)
                for s in range(2):
                    nc.tensor.matmul(psF[:, 512 * s:512 * s + 512], Wb("elu"),
                                     eluT[:, 512 * s:512 * s + 512])
                nc.scalar.activation(featA[:, c0 // 2:c0 // 2 + T],
                                     psF, AF.Identity, bias=Wc("bfeat"))
        wno += 1

        # =================== transformer ===================
        def ln_chain(sb, ps, src_ap, xn_out_ap, vbufs=2):
            """LN on one [128,T] tile: ACT evac -> fp16 DVE sq -> ones-MM ->
            AbsRsqrt -> fp16 DVE mul. src_ap is fp16 SBUF [128, T]."""
            psXC = ps.tile([128, T], dt.float32, tag="XC", bufs=2, name="psxc")
            for s in range(2):
                nc.tensor.matmul(psXC[:, 512 * s:512 * s + 512], Wb("center"),
                                 src_ap[:, 512 * s:512 * s + 512])
            xc = sb.tile([128, T], dt.float16, tag="xc", bufs=2, name="xc")
            nc.scalar.activation(xc, psXC, AF.Identity, bias=Wc("zero"))
            sq = sb.tile([128, T], dt.float16, tag="sq", bufs=2, name="sq")
            nc.vector.tensor_mul(sq, xc, xc)
            psV = ps.tile([128, T], dt.float32, tag="V", bufs=vbufs, name="psv")
            for s in range(2):
                nc.tensor.matmul(psV[:, 512 * s:512 * s + 512], Wb("ones64"),
                                 sq[:, 512 * s:512 * s + 512])
            rstd = sb.tile([128, T], dt.float16, tag="rs", bufs=2, name="rstd")
            nc.scalar.activation(rstd, psV, AF.Abs_reciprocal_sqrt,
                                 bias=Wc("eps"), scale=1.0 / 64.0)
            nc.vector.tensor_mul(xn_out_ap, xc, rstd)

        def ln_wave(src, wtag):
            nonlocal_w = wno
            with tc.tile_pool(name=f"ln{wtag}s", bufs=2) as sb, \
                 tc.tile_pool(name=f"ln{wtag}p", bufs=2, space="PSUM") as ps:
                for k in fwd(nonlocal_w):
                    sl = slice(k * T, (k + 1) * T)
                    ln_chain(sb, ps, featA[:, sl] if src is featA else featB[:, sl],
                             xn_sb[:, sl])

        for i in range(2):
            # ---- LN1 (table: abs_reciprocal_sqrt) ----
            tc.no_sync_barrier()
            ln_wave(featA, f"a{i}")
            wno += 1
            # ---- attn + residual (no ACT funcs) ----
            tc.no_sync_barrier()
            with tc.tile_pool(name=f"at{i}p", bufs=4, space="PSUM") as ps:
                for k in fwd(wno):
                    sl = slice(k * T, (k + 1) * T)
                    psF2 = ps.tile([128, T], dt.float32, tag="F2")
                    for s in range(2):
                        nc.tensor.matmul(psF2[:, 512 * s:512 * s + 512], Wb(f"attn{i}"),
                                         xn_sb[:, k * T + 512 * s:k * T + 512 * s + 512])
                    nc.vector.scalar_tensor_tensor(
                        featB[:, sl], psF2, Wc(f"bvo{i}"), featA[:, sl],
                        ALU.add, ALU.add)
            wno += 1
            # ---- LN2 ----
            tc.no_sync_barrier()
            ln_wave(featB, f"c{i}")
            wno += 1
            # ---- MLP (ACT table: gelu) ----
            tc.no_sync_barrier()
            with tc.tile_pool(name=f"ml{i}s", bufs=6) as sb, \
                 tc.tile_pool(name=f"ml{i}p", bufs=2, space="PSUM") as ps:
                for k in fwd(wno):
                    sl = slice(k * T, (k + 1) * T)
                    gsb = []
                    for q in range(4):
                        gp = ps.tile([128, T], dt.float32, tag="G", name=f"gp{q}")
                        for s in range(2):
                            nc.tensor.matmul(gp[:, 512 * s:512 * s + 512],
                                             Wb(f"mlp1q{i}_{q}"),
                                             xn_sb[:, k * T + 512 * s:k * T + 512 * s + 512])
                        g = sb.tile([128, T], dt.float16, tag=f"g{q}")
                        nc.scalar.activation(g, gp, AF.Gelu, bias=Wc(f"b1gq{i}_{q}"))
                        gsb.append(g)
                    psF3 = ps.tile([128, T], dt.float32, tag="F3")
                    for q in range(4):
                        for s in range(2):
                            nc.tensor.matmul(psF3[:, 512 * s:512 * s + 512],
                                             Wb(f"mlp2q{i}_{q}"),
                                             gsb[q][:, 512 * s:512 * s + 512],
                                             start=(q == 0), stop=(q == 3))
                    nc.vector.scalar_tensor_tensor(
                        featA[:, sl], psF3, Wc(f"b2c{i}"), featB[:, sl],
                        ALU.add, ALU.add)
            wno += 1

        # ---- final LN + classifier (abs_reciprocal_sqrt) ----
        tc.no_sync_barrier()
        with tc.tile_pool(name="clss", bufs=2) as sb, \
             tc.tile_pool(name="clsp", bufs=1, space="PSUM") as ps:
            for k in fwd(wno):
                xn3 = sb.tile([128, T], dt.float16, tag="xn3", bufs=2)
                ln_chain(sb, ps, featA[:, k * T:(k + 1) * T], xn3, vbufs=1)
                psO = ps.tile([6, T], dt.float32, tag="O", bufs=1)
                for s in range(2):
                    nc.tensor.matmul(psO[:, 512 * s:512 * s + 512], Wb("cls", n=6),
                                     xn3[:, 512 * s:512 * s + 512])
                osb = sb.tile([6, T], dt.float32, tag="osb")
                nc.vector.tensor_scalar_add(osb, psO, Wc("bcls6", 0, 6))
                nc.sync.dma_start(y_d.ap()[:, k * T:(k + 1) * T], osb)
        wno += 1

    nc.compile()
    return nc


# ---------------------------------------------------------------- entry point
def _prep_x(w):
    """Host-side: build transposed bf16 input blobs xe [320,B], xo [48,B]."""
    eeg = w["eeg"].astype(F32)                       # [B, 62, 5]
    xeT = np.zeros((320, B_TOTAL), F32)
    xeT.reshape(5, 64, B_TOTAL)[:, 0:62, :] = eeg.transpose(2, 1, 0)
    xeT[62] = 1.0
    xoT = np.zeros((48, B_TOTAL), F32)
    xoT[0:33] = w["eog"].astype(F32)[:, 0, :].T
    xoT[33] = 1.0                                    # beta bias row
    return xeT.astype(F16), xoT.astype(F16)


def _make_in_maps(w):
    wbf, wf32, wf8, cols, colsb, cols8 = _fold_weights(w)
    xeT, xoT = _prep_x(w)
    key = ("prog", wbf.shape[1], wf32.shape[1], wf8.shape[1])
    in_maps = []
    for k in range(N_CORES):
        in_maps.append({
            "xe": np.ascontiguousarray(xeT[:, k * B_CORE:(k + 1) * B_CORE]),
            "xo": np.ascontiguousarray(xoT[:, k * B_CORE:(k + 1) * B_CORE]),
            "wbf": wbf, "wf32": wf32, "wf8": wf8,
        })
    return key, in_maps, (wbf.shape[1], wf32.shape[1], wf8.shape[1], cols, colsb, cols8)


def _unshard(res):
    out = np.empty((B_TOTAL, 3), F32)
    for k in range(N_CORES):
        y = res.results[k]["y_fm"].reshape(2, 3, N_TILE, T)
        out[k * B_CORE:(k + 1) * B_CORE] = (
            y.transpose(2, 0, 3, 1).reshape(B_CORE, 3))
    return out


def kernel(**inputs):
    w = {k: np.asarray(v) for k, v in inputs.items()}
    key, in_maps, bargs = _make_in_maps(w)
    if key not in _CACHE:
        _CACHE[key] = _build(*bargs)
    nc = _CACHE[key]
    res = run_bass_kernel_spmd(nc, in_maps, core_ids=list(range(N_CORES)))
    return _unshard(res)


if __name__ == "__main__":
    import reference
    ins = {k: np.asarray(v) for k, v in reference.setup_inputs().items()}
    got = kernel(**ins)
    exp = np.asarray(reference.reference(**ins))
    err = np.abs(got - exp).max() / (np.abs(exp).max() + 1e-9)
    print("Relative error:", err)


# revision 42
# speedup vs baseline: 1.4365x; 1.1989x over previous
"""Trainium2 Bass kernel for nn_MCAF (dense_transformer).

Strategy: pure data-parallel over 8 NeuronCores (batch 131072 -> 16384/core).
v2: restructured for engine balance + PE p-state:
 - T=1024-col tiles (2 halves x 1024 batch elems = 2048 elems/tile, 8 tiles).
 - Wave structure grouped by ACT table set (exp | absrsqrt | gelu), zigzag
   tile order across waves so engines never drain at wave boundaries.
 - PSUM pools sized to exactly 8 banks per wave with >=2-deep rotation.
 - Front end: eog xo rows DMA'd straight into the y2 tile; h-conv and eog-raw
   contractions merged into one matmul chain; elu via min/exp with bf16 ops.
 - LN: center-matmul -> bf16 evac -> DVE square (4x) -> ones-matmul ->
   ACT Abs_reciprocal_sqrt -> DVE bf16 mul.
 - Classifier bias preloaded into PSUM via DMA; logits DMA'd PSUM->DRAM.
"""

import sys

sys.path.insert(0, "/opt/trn_rl_repo")

import numpy as np
import ml_dtypes

import concourse.bass as bass
import concourse.bacc as bacc
import concourse.tile as tile
from concourse import mybir
from concourse.bass_utils import run_bass_kernel_spmd

F16 = np.float16
F32 = np.float32

B_TOTAL = 131072
N_CORES = 8
B_CORE = B_TOTAL // N_CORES          # 16384
T = 1024                             # batch columns per half-tile
N_TILE = B_CORE // (2 * T)           # 8 tiles of 2 halves x T
AF = mybir.ActivationFunctionType
ALU = mybir.AluOpType
dt = mybir.dt


# ---------------------------------------------------------------- host folding
def _fold_weights(w):
    """Returns (wbf [128,NB] bf16 blob, wf32 [128,NF] f32 bias cols, maps)."""
    eeg_ow = w["eeg_ow"].astype(np.float64)
    wv = w["eeg_inw"][124:186].astype(np.float64)
    bv = w["eeg_inb"][124:186].astype(np.float64)
    Me5 = (eeg_ow @ wv) / 5.0                        # [62,62]
    c_e = eeg_ow @ bv + w["eeg_ob"].astype(np.float64)

    colsb = {}
    bblocks = []

    def addb(name, arr):
        colsb[name] = sum(b.shape[1] for b in bblocks)
        bblocks.append(arr)

    # obar blocks: x320 row g=64*l+c ; M=128 cols: [obar(62) 0 0 | obar(62) 0 0]
    for t in range(3):
        rows = 64 if t == 2 else 128
        blk = np.zeros((128, 128), np.float64)
        for r in range(rows):
            g = 128 * t + r
            c = g % 64
            if c < 62:
                blk[r, 0:62] = Me5[:, c]
                blk[r, 64:126] = Me5[:, c]
            elif g == 62:                            # host plants 1.0 in x320[:,62]
                blk[r, 0:62] = c_e
                blk[r, 64:126] = c_e
        addb(f"ob{t}", blk)

    # h blocks (eeg conv contraction): psHR rows {32h eeg-h} | raw rows later
    cw = w["eeg_cw"].astype(np.float64)              # [32,62,5]
    cwo = w["eog_cw"].astype(np.float64)             # [32,1,33]
    for t in range(3):
        rows = 64 if t == 2 else 128
        blk = np.zeros((128, 32), np.float64)
        for r in range(rows):
            g = 128 * t + r
            l, c = g // 64, g % 64
            if c < 62:
                blk[r, :] = cw[:, c, l]
        addb(f"h{t}", blk)
    # eog raw block (stored at partitions 64:112, read from y2[64:112])
    raw_blk = np.zeros((128, 32), np.float64)
    for l in range(33):
        raw_blk[64 + l, :] = cwo[:, 0, l]
    addb("ogr", raw_blk)

    # eog alpha block (stored at partitions 64:112)
    alpha33 = float(w["eog_inw"][2, 0]) * float(w["eog_ow"][0, 0]) / 33.0
    beta = float(w["eog_inb"][2]) * float(w["eog_ow"][0, 0]) + float(w["eog_ob"][0])
    alp_blk = np.zeros((128, 32), np.float64)
    alp_blk[64:64 + 33, :] = alpha33
    alp_blk[64 + 33, :] = beta                       # host plants 1.0 in xo row 33
    addb("alp", alp_blk)

    # fused (ef,of)->feat weights; psF rows [feat-h0 | feat-h1]
    fw = w["fus_w"].astype(np.float64)               # [64,128]
    W_e = fw[:, :64] @ w["eeg_fw"].astype(np.float64)    # [64,32]
    W_o = fw[:, 64:] @ w["eog_fw"].astype(np.float64)    # [64,32]
    C = np.eye(64) - 1.0 / 64.0                      # centering, folded into
    # every producer: the residual stream lives centered (C is idempotent).
    elu_blk = np.zeros((128, 128), np.float64)
    elu_blk[0:32, 0:64] = W_e.T @ C                  # eeg_h0 -> feat h0
    elu_blk[32:64, 64:128] = W_e.T @ C               # eeg_h1 -> feat h1
    elu_blk[64:96, 0:64] = W_o.T @ C                 # og_h0  -> feat h0
    elu_blk[96:128, 64:128] = W_o.T @ C              # og_h1  -> feat h1
    addb("elu", elu_blk)

    def bdiag(blk):                                  # block-diag [128,128]
        out = np.zeros((128, 128))
        out[0:64, 0:64] = blk
        out[64:128, 64:128] = blk
        return out

    addb("ones64", bdiag(np.ones((64, 64))))

    pe0 = (np.arange(64) % 2).astype(np.float64)
    b_feat = (fw[:, :64] @ w["eeg_fb"].astype(np.float64)
              + fw[:, 64:] @ w["eog_fb"].astype(np.float64)
              + w["fus_b"].astype(np.float64) + pe0
              - W_e.sum(axis=1) - W_o.sum(axis=1))   # fold elu's (e'-1)

    lay = []
    for i in range(2):
        s1 = w["tl_ln1_s"][i].astype(np.float64)
        b1v = w["tl_ln1_b"][i].astype(np.float64)
        Wvo = w["tl_ow"][i].astype(np.float64) @ w["tl_inw"][i, 128:192].astype(np.float64)
        bvo = (w["tl_ow"][i].astype(np.float64) @ w["tl_inb"][i, 128:192].astype(np.float64)
               + w["tl_ob"][i].astype(np.float64))
        Wvo_s = Wvo * s1[None, :]
        bvo_t = Wvo @ b1v + bvo
        s2 = w["tl_ln2_s"][i].astype(np.float64)
        b2v = w["tl_ln2_b"][i].astype(np.float64)
        W1 = w["tl_w1"][i].astype(np.float64)        # [256,64]
        W1_s = W1 * s2[None, :]
        b1g = W1 @ b2v + w["tl_b1"][i].astype(np.float64)   # [256]
        W2 = w["tl_w2"][i].astype(np.float64)        # [64,256]
        b2c = w["tl_b2"][i].astype(np.float64)
        lay.append((Wvo_s, bvo_t, W1_s, b1g, W2, b2c))
        addb(f"attn{i}", bdiag((C @ Wvo_s).T))
        m1 = W1_s.T                                  # [64,256]
        w2t = W2.T @ C                               # [256,64] out-centered
        for q in range(4):
            addb(f"mlp1q{i}_{q}", bdiag(m1[:, 64 * q:64 * q + 64]))
            addb(f"mlp2q{i}_{q}", bdiag(w2t[64 * q:64 * q + 64, :]))

    # fp8 DoubleRow mlp2 weights: per layer, chunk-pairs (q0,q1), (q2,q3)
    cols8 = {}
    f8blocks = []

    def add8(name, arr):
        cols8[name] = sum(b.shape[1] for b in f8blocks)
        f8blocks.append(arr)

    for i in range(2):
        w2t = lay[i][4].T                            # [256,64]
        for p in range(2):
            pair = np.concatenate(
                [bdiag(w2t[64 * (2 * p + j):64 * (2 * p + j) + 64, :])
                 for j in range(2)], axis=1)         # [128, 256]
            add8(f"mlp2d{i}_{p}", pair)
    wf8 = np.concatenate(f8blocks, axis=1)

    fn_s = w["fn_s"].astype(np.float64)
    fn_b = w["fn_b"].astype(np.float64)
    cls_w = w["cls_w"].astype(np.float64)
    cls_s = cls_w * fn_s[None, :]                    # [3,64]
    b_cls = cls_w @ fn_b + w["cls_b"].astype(np.float64)
    csT = cls_s.T                                    # [64,3]
    clsblk = np.zeros((128, 6))
    clsblk[0:64, 0:3] = csT
    clsblk[64:128, 3:6] = csT
    addb("cls", clsblk)

    wbf = np.concatenate(bblocks, axis=1)

    # --- f32 bias columns ---
    cols = {}
    blocks = []

    def add(name, arr):
        cols[name] = sum(b.shape[1] for b in blocks)
        blocks.append(arr)

    def col(vals128):
        return np.asarray(vals128, np.float64).reshape(128, 1)

    cb_e = w["eeg_cb"].astype(np.float64)
    cb_o = w["eog_cb"].astype(np.float64)
    add("cbcol", col(np.concatenate([cb_e, cb_e, cb_o, cb_o])))
    b_feat_c = C @ b_feat
    add("bfeat", col(np.concatenate([b_feat_c, b_feat_c])))
    for i in range(2):
        bvo_c = C @ lay[i][1]
        add(f"bvo{i}", col(np.concatenate([bvo_c, bvo_c])))
        for q in range(4):
            add(f"b1gq{i}_{q}", col(np.concatenate([lay[i][3][64 * q:64 * q + 64]] * 2)))
        b2c_c = C @ lay[i][5]
        add(f"b2c{i}", col(np.concatenate([b2c_c, b2c_c])))
    add("eps", col(np.full(128, 1e-5)))
    add("zero", col(np.zeros(128)))
    bc6 = np.zeros(128)
    bc6[0:3] = b_cls
    bc6[3:6] = b_cls
    add("bcls6", col(bc6))

    wf32 = np.concatenate(blocks, axis=1)
    return (wbf.astype(F16), wf32.astype(F32),
            wf8.astype(ml_dtypes.float8_e4m3fn), cols, colsb, cols8)


# ---------------------------------------------------------------- device build
_CACHE = {}


def _build(nbf, nf32, nf8, cols, colsb, cols8):
    nc = bacc.Bacc("TRN2", target_bir_lowering=False, debug=False)
    xe_d = nc.dram_tensor("xe", [320, B_CORE], dt.float16, kind="ExternalInput")
    xo_d = nc.dram_tensor("xo", [48, B_CORE], dt.float16, kind="ExternalInput")
    wbf_d = nc.dram_tensor("wbf", [128, nbf], dt.float16, kind="ExternalInput")
    wf_d = nc.dram_tensor("wf32", [128, nf32], dt.float32, kind="ExternalInput")
    wf8_d = nc.dram_tensor("wf8", [128, nf8], dt.float8e4, kind="ExternalInput")
    y_d = nc.dram_tensor("y_fm", [6, N_TILE * T], dt.float32, kind="ExternalOutput")

    # persistent sbuf arrays
    wbf_sb = nc.alloc_sbuf_tensor("wbf_sb", [128, nbf], dt.float16).ap()
    wf_sb = nc.alloc_sbuf_tensor("wf_sb", [128, nf32], dt.float32).ap()
    wf8_sb = nc.alloc_sbuf_tensor("wf8_sb", [128, nf8], dt.float8e4).ap()
    featA = nc.alloc_sbuf_tensor("featA", [128, N_TILE * T], dt.float16).ap()
    featB = nc.alloc_sbuf_tensor("featB", [128, N_TILE * T], dt.float16).ap()
    xn_sb = nc.alloc_sbuf_tensor("xn_sb", [128, N_TILE * T], dt.float16).ap()

    def Wb(name, r0=0, r1=128, n=128):
        c0 = colsb[name]
        return wbf_sb[r0:r1, c0:c0 + n]

    def W8(name):                                    # fp8 DR pair [128,2,128]
        c0 = cols8[name]
        return wf8_sb[:, c0:c0 + 256].rearrange("p (two m) -> p two m", two=2)

    def Wc(name, r0=0, r1=128):                      # bias columns
        return wf_sb[r0:r1, cols[name]:cols[name] + 1]

    def fwd(w):                                      # same order every wave:
        return range(N_TILE)                         # waves pipeline per-tile

    wno = 0

    with tile.TileContext(nc) as tc:
        nc.sync.dma_start(wbf_sb, wbf_d.ap())
        nc.sync.dma_start(wf_sb, wf_d.ap())
        nc.sync.dma_start(wf8_sb, wf8_d.ap())

        # =================== WAVE 1: front end (ACT table: exp) ===============
        with tc.tile_pool(name="w1x", bufs=2) as xp, \
             tc.tile_pool(name="w1s", bufs=2) as sb, \
             tc.tile_pool(name="w1p", bufs=1, space="PSUM") as ps:
            for k in fwd(wno):
                c0 = k * 2 * T                       # first batch col of tile
                xt = xp.tile([128, 3 * 2 * T], dt.float16, tag="xt")
                nc.sync.dma_start(xt[:, 0:2 * T], xe_d.ap()[0:128, c0:c0 + 2 * T])
                nc.sync.dma_start(xt[:, 2 * T:4 * T], xe_d.ap()[128:256, c0:c0 + 2 * T])
                nc.sync.dma_start(xt[0:64, 4 * T:6 * T], xe_d.ap()[256:320, c0:c0 + 2 * T])
                y2s = []
                for h in range(2):
                    y2 = xp.tile([112, T], dt.float16, tag=f"y2{h}")
                    nc.sync.dma_start(y2[64:112], xo_d.ap()[:, c0 + h * T:c0 + (h + 1) * T])
                    y2s.append(y2)

                psHR = ps.tile([128, T], dt.float32, tag="HR", bufs=1)
                psA = ps.tile([128, T], dt.float32, tag="A")
                for h in range(2):
                    ob2 = sb.tile([128, T], dt.float16, tag=f"ob2{h}")
                    # obar: 3 chunks x 2 slices, chunk-major for weight reuse
                    psobs = [ps.tile([128, 512], dt.float32, tag="OB", bufs=2,
                                     name=f"psob{h}{j}") for j in range(2)]
                    for t in range(3):
                        kk = 64 if t == 2 else 128
                        for s in range(2):
                            mv = xt[0:kk, 2 * T * t + h * T + 512 * s:
                                    2 * T * t + h * T + 512 * s + 512]
                            nc.tensor.matmul(psobs[s], Wb(f"ob{t}")[0:kk], mv,
                                             start=(t == 0), stop=(t == 2))
                    for s in range(2):
                        nc.scalar.activation(ob2[:, 512 * s:512 * s + 512], psobs[s],
                                             AF.Identity, bias=Wc("zero"))
                    # y = x * obar  (bf16, 4x DVE)
                    y0 = sb.tile([128, T], dt.float16, tag=f"y0{h}")
                    y1 = sb.tile([128, T], dt.float16, tag=f"y1{h}")
                    nc.vector.tensor_mul(y0, xt[:, h * T:(h + 1) * T], ob2)
                    nc.vector.tensor_mul(y1, xt[:, 2 * T + h * T:2 * T + (h + 1) * T], ob2)
                    nc.vector.tensor_mul(y2s[h][0:64],
                                         xt[0:64, 4 * T + h * T:4 * T + (h + 1) * T],
                                         ob2[0:64])
                    # eeg h chain -> psHR rows 32h:32h+32
                    for t in range(3):
                        kk = 64 if t == 2 else 128
                        for s in range(2):
                            if t == 2:
                                mv = y2s[h][0:64, 512 * s:512 * s + 512]
                            else:
                                yy = y0 if t == 0 else y1
                                mv = yy[:, 512 * s:512 * s + 512]
                            nc.tensor.matmul(psHR[32 * h:32 * h + 32,
                                                  512 * s:512 * s + 512],
                                             Wb(f"h{t}", 0, kk, n=32), mv,
                                             start=(t == 0), stop=(t == 2))
                # eog alpha / raw MMs (psR reuses the OB psum ring)
                psRs = [ps.tile([128, 512], dt.float32, tag="OB", bufs=2,
                                name=f"psr{j}") for j in range(2)]
                for h in range(2):
                    o2 = 64 + 32 * h
                    for s in range(2):
                        nc.tensor.matmul(psA[o2:o2 + 32, 512 * s:512 * s + 512],
                                         Wb("alp", 64, 112, n=32),
                                         y2s[h][64:112, 512 * s:512 * s + 512],
                                         tile_position=(64, o2))
                        nc.tensor.matmul(psRs[s][o2:o2 + 32],
                                         Wb("ogr", 64, 112, n=32),
                                         y2s[h][64:112, 512 * s:512 * s + 512],
                                         tile_position=(64, o2))
                # og = raw * (alpha*mean + beta): evac alpha, mult into psHR
                o2a = sb.tile([128, T], dt.float16, tag="o2a")
                nc.scalar.activation(o2a[64:128], psA[64:128], AF.Identity,
                                     bias=Wc("zero", 64, 128))
                for s in range(2):
                    nc.vector.tensor_mul(psHR[64:128, 512 * s:512 * s + 512],
                                         psRs[s][64:128],
                                         o2a[64:128, 512 * s:512 * s + 512])
                # elu(z)+1 = max(z,0) + exp(min(z,0)); -1 folded into bfeat
                r1 = sb.tile([128, T], dt.float16, tag="r1")
                sm = sb.tile([128, T], dt.float16, tag="sm")
                e1 = sb.tile([128, T], dt.float16, tag="e1")
                eluT = sb.tile([128, T], dt.float16, tag="elu")
                nc.scalar.activation(r1, psHR, AF.Relu, bias=Wc("cbcol"))
                nc.vector.tensor_scalar(sm, psHR, Wc("cbcol"), 0.0, ALU.add, ALU.min)
                nc.scalar.activation(e1, sm, AF.Exp, bias=Wc("zero"))
                nc.vector.tensor_add(eluT, r1, e1)
                # feat = W_elu.T @ elu (+b_feat)
                psF = ps.tile([128, T], dt.float32, tag="F", bufs=1)
                for s in range(2):
                    nc.tensor.matmul(psF[:, 512 * s:512 * s + 512], Wb("elu"),
                                     eluT[:, 512 * s:512 * s + 512])
                nc.scalar.activation(featA[:, c0 // 2:c0 // 2 + T],
                                     psF, AF.Identity, bias=Wc("bfeat"))
        wno += 1

        # =================== transformer ===================
        def ln_chain(sb, ps, src_ap, xn_out_ap, vbufs=4):
            """LN on a centered fp16 SBUF [128,T] tile: DVE sq (4x) ->
            ones-MM -> AbsRsqrt -> DVE mul (4x). No centering needed."""
            sq = sb.tile([128, T], dt.float16, tag="sq", bufs=3, name="sq")
            nc.vector.tensor_mul(sq, src_ap, src_ap)
            psV = ps.tile([128, T], dt.float32, tag="V", bufs=vbufs, name="psv")
            for s in range(2):
                nc.tensor.matmul(psV[:, 512 * s:512 * s + 512], Wb("ones64"),
                                 sq[:, 512 * s:512 * s + 512])
            rstd = sb.tile([128, T], dt.float16, tag="rs", bufs=3, name="rstd")
            nc.scalar.activation(rstd, psV, AF.Abs_reciprocal_sqrt,
                                 bias=Wc("eps"), scale=1.0 / 64.0)
            nc.vector.tensor_mul(xn_out_ap, src_ap, rstd)

        def ln_wave(src, wtag):
            nonlocal_w = wno
            with tc.tile_pool(name=f"ln{wtag}s", bufs=2) as sb, \
                 tc.tile_pool(name=f"ln{wtag}p", bufs=2, space="PSUM") as ps:
                for k in fwd(nonlocal_w):
                    sl = slice(k * T, (k + 1) * T)
                    ln_chain(sb, ps, featA[:, sl] if src is featA else featB[:, sl],
                             xn_sb[:, sl])

        for i in range(2):
            # ---- LN1 (table: abs_reciprocal_sqrt) ----
            tc.no_sync_barrier()
            ln_wave(featA, f"a{i}")
            wno += 1
            # ---- attn + residual (no ACT funcs) ----
            tc.no_sync_barrier()
            with tc.tile_pool(name=f"at{i}p", bufs=4, space="PSUM") as ps:
                for k in fwd(wno):
                    sl = slice(k * T, (k + 1) * T)
                    psF2 = ps.tile([128, T], dt.float32, tag="F2")
                    for s in range(2):
                        nc.tensor.matmul(psF2[:, 512 * s:512 * s + 512], Wb(f"attn{i}"),
                                         xn_sb[:, k * T + 512 * s:k * T + 512 * s + 512])
                    nc.vector.scalar_tensor_tensor(
                        featB[:, sl], psF2, Wc(f"bvo{i}"), featA[:, sl],
                        ALU.add, ALU.add)
            wno += 1
            # ---- LN2 ----
            tc.no_sync_barrier()
            ln_wave(featB, f"c{i}")
            wno += 1
            # ---- MLP (ACT table: gelu) ----
            tc.no_sync_barrier()
            with tc.tile_pool(name=f"ml{i}s", bufs=6) as sb, \
                 tc.tile_pool(name=f"ml{i}p", bufs=2, space="PSUM") as ps:
                for k in fwd(wno):
                    sl = slice(k * T, (k + 1) * T)
                    gsb = []
                    for q in range(4):
                        gp = ps.tile([128, T], dt.float32, tag="G", name=f"gp{q}")
                        for s in range(2):
                            nc.tensor.matmul(gp[:, 512 * s:512 * s + 512],
                                             Wb(f"mlp1q{i}_{q}"),
                                             xn_sb[:, k * T + 512 * s:k * T + 512 * s + 512])
                        g = sb.tile([128, T], dt.float16, tag=f"g{q}")
                        nc.scalar.activation(g, gp, AF.Gelu, bias=Wc(f"b1gq{i}_{q}"))
                        gsb.append(g)
                    psF3 = ps.tile([128, T], dt.float32, tag="F3")
                    for q in range(4):
                        for s in range(2):
                            nc.tensor.matmul(psF3[:, 512 * s:512 * s + 512],
                                             Wb(f"mlp2q{i}_{q}"),
                                             gsb[q][:, 512 * s:512 * s + 512],
                                             start=(q == 0), stop=(q == 3))
                    nc.vector.scalar_tensor_tensor(
                        featA[:, sl], psF3, Wc(f"b2c{i}"), featB[:, sl],
                        ALU.add, ALU.add)
            wno += 1

        # ---- final LN + classifier (abs_reciprocal_sqrt) ----
        tc.no_sync_barrier()
        with tc.tile_pool(name="clss", bufs=2) as sb, \
             tc.tile_pool(name="clsp", bufs=1, space="PSUM") as ps:
            for k in fwd(wno):
                xn3 = sb.tile([128, T], dt.float16, tag="xn3", bufs=2)
                ln_chain(sb, ps, featA[:, k * T:(k + 1) * T], xn3, vbufs=2)
                psO = ps.tile([6, T], dt.float32, tag="O", bufs=1)
                for s in range(2):
                    nc.tensor.matmul(psO[:, 512 * s:512 * s + 512], Wb("cls", n=6),
                                     xn3[:, 512 * s:512 * s + 512])
                osb = sb.tile([6, T], dt.float32, tag="osb")
                nc.vector.tensor_scalar_add(osb, psO, Wc("bcls6", 0, 6))
                nc.sync.dma_start(y_d.ap()[:, k * T:(k + 1) * T], osb)
        wno += 1

    nc.compile()
    return nc


# ---------------------------------------------------------------- entry point
def _prep_x(w):
    """Host-side: build transposed bf16 input blobs xe [320,B], xo [48,B]."""
    eeg = w["eeg"].astype(F32)                       # [B, 62, 5]
    xeT = np.zeros((320, B_TOTAL), F32)
    xeT.reshape(5, 64, B_TOTAL)[:, 0:62, :] = eeg.transpose(2, 1, 0)
    xeT[62] = 1.0
    xoT = np.zeros((48, B_TOTAL), F32)
    xoT[0:33] = w["eog"].astype(F32)[:, 0, :].T
    xoT[33] = 1.0                                    # beta bias row
    return xeT.astype(F16), xoT.astype(F16)


def _make_in_maps(w):
    wbf, wf32, wf8, cols, colsb, cols8 = _fold_weights(w)
    xeT, xoT = _prep_x(w)
    key = ("prog", wbf.shape[1], wf32.shape[1], wf8.shape[1])
    in_maps = []
    for k in range(N_CORES):
        in_maps.append({
            "xe": np.ascontiguousarray(xeT[:, k * B_CORE:(k + 1) * B_CORE]),
            "xo": np.ascontiguousarray(xoT[:, k * B_CORE:(k + 1) * B_CORE]),
            "wbf": wbf, "wf32": wf32, "wf8": wf8,
        })
    return key, in_maps, (wbf.shape[1], wf32.shape[1], wf8.shape[1], cols, colsb, cols8)


def _unshard(res):
    out = np.empty((B_TOTAL, 3), F32)
    for k in range(N_CORES):
        y = res.results[k]["y_fm"].reshape(2, 3, N_TILE, T)
        out[k * B_CORE:(k + 1) * B_CORE] = (
            y.transpose(2, 0, 3, 1).reshape(B_CORE, 3))
    return out


def kernel(**inputs):
    w = {k: np.asarray(v) for k, v in inputs.items()}
    key, in_maps, bargs = _make_in_maps(w)
    if key not in _CACHE:
        _CACHE[key] = _build(*bargs)
    nc = _CACHE[key]
    res = run_bass_kernel_spmd(nc, in_maps, core_ids=list(range(N_CORES)))
    return _unshard(res)


if __name__ == "__main__":
    import reference
    ins = {k: np.asarray(v) for k, v in reference.setup_inputs().items()}
    got = kernel(**ins)
    exp = np.asarray(reference.reference(**ins))
    err = np.abs(got - exp).max() / (np.abs(exp).max() + 1e-9)
    print("Relative error:", err)
